# revision 9
# baseline (speedup 1.0000x reference)
"""Trainium2 Bass kernel for nn_PolicyNetwork3 (2-layer GraphSAGE + edge-MLP).

Design (8 NeuronCores, SPMD single NEFF):
- dst-sharded aggregation: core k owns node block [6272k, 6272k+6272).
- Edges sorted by (core, 512-dst window, src-half); gathered from HBM row
  tables via dma_gather (int16 idx -> LO/HI table halves of 25088 rows).
  Trailing pad indices are -1 so the Q7 desc-gen trims them.
- segment-sum per 512-dst window via one-hot matmuls accumulating into a
  full PSUM bank (one-hot weighted by 1/max(deg,1), built on DVE).
- Linear+BN (folded) emitted per window, flipped to produce [feat, node]
  so no transpose is needed for the next layer's input; layer-1 fuses the
  candidate-MLP first-layer projection (gq tables) into the block loop.
- h shard exchanged after layer 0 with an AllGather; candidate MLP runs
  in 512-candidate tiles pipelined with the u/v gathers; global softmax
  on-device after an AllGather of y.
"""

import sys

sys.path.insert(0, "/opt/trn_rl_repo")
sys.path.insert(0, "/root/.axon_site")

import numpy as np

import concourse.bacc as bacc
import concourse.bass as bass
import concourse.bass_isa as bass_isa
import concourse.mybir as mybir
import concourse.tile as tile
from concourse import library_config
from concourse.bass_utils import run_bass_kernel_spmd

P = 128
N, E, C = 50000, 800000, 100000
D = 128
NCORE = 8
NSH = 6272          # nodes per core shard
NTOT = NSH * NCORE  # 50176 padded node table
WINW = 512          # aggregation window width (one PSUM bank)
NWIN = -(-NSH // WINW)  # 13 windows (12 full + 1 of 128 dsts)
HALF = NTOT // 2    # 25088 rows per gather-table half
CSH = C // NCORE    # 12500 candidates per core
GCALL = 2048        # max idxs per dma_gather call
TGRP = 4            # candidate chunks per MLP tile (512 cands)
BN_EPS = 1e-5
SLOPE = 0.01
F32 = mybir.dt.float32
I16 = mybir.dt.int16
AF = mybir.ActivationFunctionType
ALU = mybir.AluOpType


def _wrap16(idx_lin):
    """[n] -> [128, n/16] int16 in the dma_gather wrapped+replicated layout."""
    n = idx_lin.shape[0]
    assert n % 16 == 0
    w = idx_lin.reshape(n // 16, 16).T.astype(np.int16)
    return np.tile(w, (8, 1)).copy()


def gidx_to_cols(arr):
    """[nslot] -> [128, nchunk] with slot i at [i%128, i//128]."""
    n = arr.shape[0]
    return arr.reshape(n // P, P).T.copy()


def _prep_edges(src, dst, invdeg):
    """Per-window (LO,HI) runs of 128-edge chunks, uniform across cores."""
    core = np.minimum(dst // NSH, NCORE - 1)
    local = dst - core * NSH
    winl = local // WINW
    half = (src >= HALF).astype(np.int64)
    key = (core * NWIN + winl) * 2 + half
    order = np.argsort(key, kind="stable")
    cnt = np.bincount(key, minlength=NCORE * NWIN * 2).reshape(NCORE, NWIN * 2)
    nch_u = (-(-cnt // P)).max(axis=0)       # [NWIN*2] uniform chunk counts
    runs = [(w, s) for w in range(NWIN) for s in (0, 1)]
    run_nch = [int(nch_u[w * 2 + s]) for (w, s) in runs]
    run_off = np.zeros(len(runs) + 1, np.int64)
    np.cumsum(run_nch, out=run_off[1:])
    tot_ch = int(run_off[-1])
    nslot = tot_ch * P

    gidx = np.zeros((NCORE, nslot), np.int16)        # pads gather row 0
    dstloc = np.full((NCORE, nslot), -5.0, np.float32)
    val = np.zeros((NCORE, nslot), np.float32)
    bstart = np.zeros(NCORE * NWIN * 2 + 1, np.int64)
    np.cumsum(np.bincount(key, minlength=NCORE * NWIN * 2), out=bstart[1:])
    for k in range(NCORE):
        for ri, (w, s) in enumerate(runs):
            b = (k * NWIN + w) * 2 + s
            e0, e1 = bstart[b], bstart[b + 1]
            n = e1 - e0
            if n == 0:
                continue
            sl = order[e0:e1]
            sl = sl[np.argsort(src[sl], kind="stable")]
            pos = int(run_off[ri]) * P
            gidx[k, pos : pos + n] = (src[sl] - s * HALF).astype(np.int16)
            dstloc[k, pos : pos + n] = (dst[sl] - k * NSH - w * WINW).astype(np.float32)
            val[k, pos : pos + n] = invdeg[dst[sl]]
    # per-run gather calls (slot_start, n_idx, half); <= GCALL idx each
    run_calls = []
    for ri, (w, s) in enumerate(runs):
        p0, p1 = int(run_off[ri]) * P, int(run_off[ri + 1]) * P
        calls = []
        p = p0
        while p < p1:
            n = min(GCALL, p1 - p)
            calls.append((p, n, s))
            p += n
        run_calls.append(calls)
    meta = dict(runs=runs, run_nch=run_nch, run_off=run_off, tot_ch=tot_ch,
                nslot=nslot, run_calls=run_calls)
    data = [dict(gidx=_wrap16(gidx[k]),
                 dstloc=gidx_to_cols(dstloc[k]),
                 val=gidx_to_cols(val[k])) for k in range(NCORE)]
    return meta, data


def _prep_cands(cand_u, cand_v, cand_feat):
    """Shard candidates, group by (u_half, v_half), pad to uniform chunks."""
    percore = [np.arange(k * CSH, (k + 1) * CSH) for k in range(NCORE)]
    groups = [[None] * 4 for _ in range(NCORE)]
    for k in range(NCORE):
        ids = percore[k]
        g = (cand_u[ids] >= HALF) * 2 + (cand_v[ids] >= HALF)
        o = np.argsort(g, kind="stable")
        ids = ids[o]
        gs = g[o]
        for gi in range(4):
            gids = ids[gs == gi]
            groups[k][gi] = gids[np.argsort(cand_u[gids], kind="stable")]
    gch = np.zeros((NCORE, 4), np.int64)
    for k in range(NCORE):
        for gi in range(4):
            gch[k, gi] = -(-len(groups[k][gi]) // P)
    gch_u = gch.max(axis=0)                 # uniform chunks per group
    ncc = int(gch_u.sum())
    cslot = ncc * P
    cu = np.zeros((NCORE, cslot), np.int16)
    cv = np.zeros((NCORE, cslot), np.int16)
    ft = np.zeros((NCORE, cslot), np.float32)
    mask = np.full((NCORE, cslot), -1e30, np.float32)
    slotmap = np.full((NCORE, cslot), -1, np.int64)
    goff = np.zeros(5, np.int64)
    np.cumsum(gch_u * P, out=goff[1:])
    for k in range(NCORE):
        for gi in range(4):
            ids = groups[k][gi]
            n = len(ids)
            p0 = goff[gi]
            uh, vh = gi // 2, gi % 2
            cu[k, p0 : p0 + n] = (cand_u[ids] - uh * HALF).astype(np.int16)
            cv[k, p0 : p0 + n] = (cand_v[ids] - vh * HALF).astype(np.int16)
            ft[k, p0 : p0 + n] = cand_feat[ids, 0]
            mask[k, p0 : p0 + n] = 0.0
            slotmap[k, p0 : p0 + n] = ids
    # gather calls: u -> runs (groups 0-1 | 2-3); v -> one run per group
    ucalls, vcalls = [], []
    for s, lo, hi in ((0, goff[0], goff[2]), (1, goff[2], goff[4])):
        p = lo
        while p < hi:
            n = min(GCALL, hi - p)
            ucalls.append((int(p), int(n), s))
            p += n
    for gi in range(4):
        p, hi = goff[gi], goff[gi + 1]
        while p < hi:
            n = min(GCALL, hi - p)
            vcalls.append((int(p), int(n), gi % 2))
            p += n
    meta = dict(ncc=ncc, cslot=cslot, ucalls=ucalls, vcalls=vcalls)
    data = [dict(cu=_wrap16(cu[k]), cv=_wrap16(cv[k]),
                 feat=gidx_to_cols(ft[k]), mask=gidx_to_cols(mask[k]),
                 slotmap=slotmap[k]) for k in range(NCORE)]
    return meta, data


def _build_nc(em, cm):
    nc = bacc.Bacc("TRN2", target_bir_lowering=False, debug=False,
                   num_devices=NCORE)
    f32 = F32
    TOTCH, NSLOT = em["tot_ch"], em["nslot"]
    NCC, CSLOT = cm["ncc"], cm["cslot"]

    # ---- external inputs ----
    xpad = nc.dram_tensor("xpad", [NTOT, D], f32, kind="ExternalInput")
    xT = nc.dram_tensor("xT", [P, NSH], f32, kind="ExternalInput")
    gidx = nc.dram_tensor("gidx", [P, NSLOT // 16], I16, kind="ExternalInput")
    dstloc = nc.dram_tensor("dstloc", [P, TOTCH], f32, kind="ExternalInput")
    val = nc.dram_tensor("val", [P, TOTCH], f32, kind="ExternalInput")
    wself = [nc.dram_tensor(f"wself{l}", [D, D], f32, kind="ExternalInput") for l in range(2)]
    wneigh = [nc.dram_tensor(f"wneigh{l}", [D, D], f32, kind="ExternalInput") for l in range(2)]
    crow = [nc.dram_tensor(f"crow{l}", [1, D], f32, kind="ExternalInput") for l in range(2)]
    iota = nc.dram_tensor("iota", [P, WINW], f32, kind="ExternalInput")
    ident = nc.dram_tensor("ident", [P, P], f32, kind="ExternalInput")
    onesr = nc.dram_tensor("onesr", [1, P], f32, kind="ExternalInput")
    abmat = nc.dram_tensor("abmat", [D, D], f32, kind="ExternalInput")
    gqbias = nc.dram_tensor("gqbias", [1, D], f32, kind="ExternalInput")
    mw0r = nc.dram_tensor("mw0r", [P, 64], f32, kind="ExternalInput")
    mw1b = nc.dram_tensor("mw1b", [65, 64], f32, kind="ExternalInput")
    mw2b = nc.dram_tensor("mw2b", [65, 1], f32, kind="ExternalInput")
    cu = nc.dram_tensor("cu", [P, CSLOT // 16], I16, kind="ExternalInput")
    cv = nc.dram_tensor("cv", [P, CSLOT // 16], I16, kind="ExternalInput")
    feat = nc.dram_tensor("feat", [P, NCC], f32, kind="ExternalInput")
    maskr = nc.dram_tensor("maskr", [P, NCC], f32, kind="ExternalInput")
    # ---- outputs ----
    y_out = nc.dram_tensor("y_out", [P, NCC], f32, kind="ExternalOutput")
    p_out = nc.dram_tensor("p_out", [P, NCORE * CSLOT // P], f32, kind="ExternalOutput")
    # ---- internal DRAM ----
    hsh0 = nc.dram_tensor("hsh0", [NSH, D], f32, kind="Internal")
    hfull = nc.dram_tensor("hfull", [NTOT, D], f32, kind="Internal", addr_space="Shared")
    gqsh = nc.dram_tensor("gqsh", [NSH, D], f32, kind="Internal")
    gqfull = nc.dram_tensor("gqfull", [NTOT, D], f32, kind="Internal", addr_space="Shared")
    ysh = nc.dram_tensor("ysh", [P, NCC], f32, kind="Internal")
    yfull = nc.dram_tensor("yfull", [NCORE * P, NCC], f32, kind="Internal", addr_space="Shared")

    rg = [list(range(NCORE))]
    runs, run_nch, run_calls = em["runs"], em["run_nch"], em["run_calls"]

    with tile.TileContext(nc) as tc:
        with (
            tc.tile_pool(name="const", bufs=1) as cp,
            tc.tile_pool(name="big", bufs=1) as bp,
            tc.tile_pool(name="msgs", bufs=3) as mp,
            tc.tile_pool(name="oh", bufs=4) as ohp,
            tc.tile_pool(name="wrk", bufs=4) as wp,
            tc.tile_pool(name="zt", bufs=3) as zp,
            tc.tile_pool(name="ps_run", bufs=2, space="PSUM") as ps_run,
            tc.tile_pool(name="ps_t", bufs=2, space="PSUM") as ps_t,
            tc.tile_pool(name="ps_h", bufs=2, space="PSUM") as ps_h,
        ):
            nc.gpsimd.load_library(library_config.mlp)

            def load(pool, t, shape=None):
                tl = pool.tile(shape or list(t.shape), t.dtype, tag=t.name)
                nc.sync.dma_start(tl[:], t[:])
                return tl

            gidx_t = load(cp, gidx)
            dstloc_t = load(cp, dstloc)
            val_t = load(cp, val)
            iota_t = load(cp, iota)
            ident_t = load(cp, ident)
            onesr_t = load(cp, onesr)
            wself_t = [load(cp, w) for w in wself]
            wneigh_t = [load(cp, w) for w in wneigh]
            crow_t = [load(cp, w) for w in crow]
            abmat_t = load(cp, abmat)
            gqbias_t = load(cp, gqbias)
            mw0r_t = load(cp, mw0r)
            mw1b_t = load(cp, mw1b)
            mw2b_t = load(cp, mw2b)
            cu_t = load(cp, cu)
            cv_t = load(cp, cv)
            feat_t = load(cp, feat)
            mask_t = load(cp, maskr)

            curT = bp.tile([P, NSH], f32, tag="curT")
            nxtT = bp.tile([P, NSH], f32, tag="nxtT")
            nc.sync.dma_start(curT[:], xT[:])

            for layer in range(2):
                gtab = xpad if layer == 0 else hfull
                for w in range(NWIN):
                    nblk = min(4, (NSH - w * WINW) // P)
                    wch = run_nch[2 * w] + run_nch[2 * w + 1]
                    aggw = wp.tile([P, WINW], f32, tag="aggw")
                    if wch == 0:
                        nc.vector.memset(aggw[:], 0.0)
                    else:
                        ps = ps_run.tile([P, WINW], f32, tag="psw")
                        jw = 0
                        for s in (0, 1):
                            for (s0, n_idx, sh) in run_calls[2 * w + s]:
                                ncall = n_idx // P
                                g = mp.tile([P, GCALL // P, P], f32, tag="g")
                                nc.gpsimd.dma_gather(
                                    g[:, :ncall, :],
                                    gtab[sh * HALF : (sh + 1) * HALF, :],
                                    gidx_t[:, s0 // 16 : (s0 + n_idx) // 16],
                                    n_idx, n_idx, P, single_packet=False)
                                for cc in range(ncall):
                                    ch = s0 // P + cc
                                    oh = ohp.tile([P, WINW], f32, tag="oh")
                                    nc.vector.tensor_scalar(
                                        oh[:], iota_t[:], dstloc_t[:, ch : ch + 1],
                                        val_t[:, ch : ch + 1], ALU.is_equal, ALU.mult)
                                    nc.tensor.matmul(ps[:], lhsT=g[:, cc, :], rhs=oh[:],
                                                     start=(jw == 0), stop=(jw == wch - 1))
                                    jw += 1
                        nc.scalar.activation(aggw[:], ps[:], AF.Copy)
                    # fused linear + BN + leaky per 128-node block
                    for b in range(nblk):
                        nb = w * WINW + b * P
                        ph = ps_h.tile([P, P], f32, tag="ph")
                        nc.tensor.matmul(ph[:], lhsT=wneigh_t[layer][:],
                                         rhs=aggw[:, b * P : (b + 1) * P],
                                         start=True, stop=False)
                        nc.tensor.matmul(ph[:], lhsT=wself_t[layer][:],
                                         rhs=curT[:, nb : nb + P], start=False, stop=False)
                        nc.tensor.matmul(ph[:], lhsT=crow_t[layer][:], rhs=onesr_t[:],
                                         start=False, stop=True)
                        tmp = wp.tile([P, P], f32, tag="tmp")
                        nc.vector.tensor_scalar(tmp[:], ph[:], SLOPE, None, ALU.mult)
                        nc.vector.tensor_tensor(nxtT[:, nb : nb + P], ph[:], tmp[:], ALU.max)
                        if layer == 0:
                            pt = ps_t.tile([P, P], f32, tag="pt")
                            nc.tensor.transpose(pt[:], nxtT[:, nb : nb + P], ident_t[:])
                            hb = wp.tile([P, P], f32, tag="hb")
                            nc.scalar.activation(hb[:], pt[:], AF.Copy)
                            nc.sync.dma_start(hsh0[nb : nb + P, :], hb[:])
                        else:
                            pg = ps_h.tile([P, P], f32, tag="ph")
                            nc.tensor.matmul(pg[:], lhsT=nxtT[:, nb : nb + P],
                                             rhs=abmat_t[:], start=True, stop=False)
                            nc.tensor.matmul(pg[:], lhsT=onesr_t[:], rhs=gqbias_t[:],
                                             start=False, stop=True)
                            gb = wp.tile([P, P], f32, tag="gb")
                            nc.scalar.activation(gb[:], pg[:], AF.Copy)
                            nc.sync.dma_start(gqsh[nb : nb + P, :], gb[:])
                if layer == 0:
                    nc.gpsimd.collective_compute(
                        "AllGather", ALU.bypass, replica_groups=rg,
                        ins=[hsh0[:].opt()], outs=[hfull[:].opt()])
                    curT, nxtT = nxtT, curT

            nc.gpsimd.collective_compute(
                "AllGather", ALU.bypass, replica_groups=rg,
                ins=[gqsh[:].opt()], outs=[gqfull[:].opt()])

            # ---- candidate gathers (u then v, interleaved by slot) ----
            ut = bp.tile([P, NCC, 64], f32, tag="ut")
            vt = bp.tile([P, NCC, 64], f32, tag="vt")
            merged = sorted(
                [(s0, n, s, 0) for (s0, n, s) in cm["ucalls"]]
                + [(s0, n, s, 1) for (s0, n, s) in cm["vcalls"]],
                key=lambda t: (t[0], t[3]))
            for (s0, n_idx, s, which) in merged:
                tl, idx_t, cofs = ((ut, cu_t, 0), (vt, cv_t, 64))[which]
                nc.gpsimd.dma_gather(
                    tl[:, s0 // P : (s0 + n_idx) // P, :],
                    gqfull[s * HALF : (s + 1) * HALF, cofs : cofs + 64],
                    idx_t[:, s0 // 16 : (s0 + n_idx) // 16],
                    n_idx, n_idx, 64, elem_step=P, single_packet=False)

            # ---- candidate MLP in 512-cand tiles ----
            ycol = bp.tile([P, NCC], f32, tag="ycol")
            for t0 in range(0, NCC, TGRP):
                tn = min(TGRP, NCC - t0)
                z1 = zp.tile([P, TGRP, 64], f32, tag="z1")
                for c in range(tn):
                    nc.vector.tensor_scalar(z1[:, c, :], mw0r_t[:],
                                            feat_t[:, t0 + c : t0 + c + 1],
                                            None, ALU.mult)
                nc.vector.tensor_tensor(z1[:, :tn, :], z1[:, :tn, :],
                                        ut[:, t0 : t0 + tn, :], ALU.add)
                nc.vector.tensor_tensor(z1[:, :tn, :], z1[:, :tn, :],
                                        vt[:, t0 : t0 + tn, :], ALU.add)
                zs = zp.tile([P, TGRP, 64], f32, tag="zs")
                nc.vector.tensor_scalar(zs[:, :tn, :], z1[:, :tn, :], SLOPE,
                                        None, ALU.mult)
                nc.vector.tensor_tensor(z1[:, :tn, :], z1[:, :tn, :],
                                        zs[:, :tn, :], ALU.max)
                z1t = zp.tile([65, TGRP * P], f32, tag="z1t")
                nc.vector.memset(z1t[64:65, :], 1.0)
                for c in range(tn):
                    pt2 = ps_t.tile([64, P], f32, tag="pt")
                    nc.tensor.transpose(pt2[:], z1[:, c, :], ident_t[:])
                    nc.scalar.activation(z1t[0:64, c * P : (c + 1) * P], pt2[:], AF.Copy)
                ps2 = ps_run.tile([64, TGRP * P], f32, tag="ps2")
                nc.tensor.matmul(ps2[:, : tn * P], lhsT=mw1b_t[:],
                                 rhs=z1t[:, : tn * P], start=True, stop=True)
                z2t = zp.tile([65, TGRP * P], f32, tag="z2t")
                nc.vector.memset(z2t[64:65, :], 1.0)
                nc.vector.tensor_scalar(z2t[0:64, : tn * P], ps2[:, : tn * P],
                                        SLOPE, None, ALU.mult)
                nc.vector.tensor_tensor(z2t[0:64, : tn * P], z2t[0:64, : tn * P],
                                        ps2[:, : tn * P], ALU.max)
                for c in range(tn):
                    py = ps_h.tile([P, 1], f32, tag="ph")
                    nc.tensor.matmul(py[:], lhsT=z2t[:, c * P : (c + 1) * P],
                                     rhs=mw2b_t[:], start=True, stop=True)
                    nc.scalar.activation(ycol[:, t0 + c : t0 + c + 1], py[:], AF.Copy)

            nc.sync.dma_start(y_out[:], ycol[:])
            ym = wp.tile([P, NCC], f32, tag="ym")
            nc.vector.tensor_tensor(ym[:], ycol[:], mask_t[:], ALU.add)
            nc.sync.dma_start(ysh[:], ym[:])
            nc.gpsimd.collective_compute(
                "AllGather", ALU.bypass, replica_groups=rg,
                ins=[ysh[:].opt()], outs=[yfull[:].opt()])
            # ---- softmax ----
            ncols = NCORE * CSLOT // P
            yf = bp.tile([P, ncols], f32, tag="yf")
            nc.sync.dma_start(yf[:], yfull[:].rearrange("a b -> (a b)")
                              .rearrange("(p c) -> p c", p=P))
            rmax = wp.tile([P, 1], f32, tag="rmax")
            nc.vector.tensor_reduce(rmax[:], yf[:], mybir.AxisListType.X, ALU.max)
            gmax = wp.tile([P, 1], f32, tag="gmax")
            nc.gpsimd.partition_all_reduce(gmax[:], rmax[:], P,
                                           bass_isa.ReduceOp.max)
            ngmax = wp.tile([P, 1], f32, tag="ngmax")
            nc.vector.tensor_scalar(ngmax[:], gmax[:], -1.0, None, ALU.mult)
            ef = bp.tile([P, ncols], f32, tag="ef")
            se = wp.tile([P, 1], f32, tag="se")
            nc.scalar.activation(ef[:], yf[:], AF.Exp, bias=ngmax[:, 0:1],
                                 accum_out=se[:])
            stot = wp.tile([P, 1], f32, tag="stot")
            nc.gpsimd.partition_all_reduce(stot[:], se[:], P, bass_isa.ReduceOp.add)
            invs = wp.tile([P, 1], f32, tag="invs")
            nc.vector.reciprocal(invs[:], stot[:])
            pf = bp.tile([P, ncols], f32, tag="pf")
            nc.vector.tensor_scalar(pf[:], ef[:], invs[:, 0:1], None, ALU.mult)
            nc.sync.dma_start(p_out[:], pf[:])
    nc.compile()
    return nc


def kernel(x, src, dst, cand_u, cand_v, cand_feat,
           w_self0, w_neigh0, b0, gamma0, beta0, rm0, rv0,
           w_self1, w_neigh1, b1, gamma1, beta1, rm1, rv1,
           mw0, mb0, mw1, mb1, mw2, mb2):
    x = np.asarray(x, np.float32)
    src = np.asarray(src, np.int64)
    dst = np.asarray(dst, np.int64)
    cand_u = np.asarray(cand_u, np.int64)
    cand_v = np.asarray(cand_v, np.int64)
    cand_feat = np.asarray(cand_feat, np.float32)

    deg = np.bincount(dst, minlength=N).astype(np.float32)
    invdeg = 1.0 / np.maximum(deg, 1.0)
    em, edata = _prep_edges(src, dst, invdeg)
    cm, cdata = _prep_cands(cand_u, cand_v, cand_feat)

    xpad = np.zeros((NTOT, D), np.float32)
    xpad[:N] = x
    iota = np.tile(np.arange(WINW, dtype=np.float32), (P, 1))
    ident = np.eye(P, dtype=np.float32)
    onesr = np.ones((1, P), np.float32)

    com = {"xpad": xpad, "iota": iota, "ident": ident, "onesr": onesr}
    for l, (ws, wn, b, ga, be, rme, rve) in enumerate(
        ((w_self0, w_neigh0, b0, gamma0, beta0, rm0, rv0),
         (w_self1, w_neigh1, b1, gamma1, beta1, rm1, rv1))):
        a = (ga / np.sqrt(rve + BN_EPS)).astype(np.float32)
        com[f"wself{l}"] = (ws * a[None, :]).astype(np.float32)
        com[f"wneigh{l}"] = (wn * a[None, :]).astype(np.float32)
        com[f"crow{l}"] = (a * (b - rme) + be).astype(np.float32)[None, :]
    com["abmat"] = np.concatenate(
        [np.asarray(mw0[0:128], np.float32), np.asarray(mw0[128:256], np.float32)], axis=1)
    com["gqbias"] = np.concatenate(
        [np.zeros(64, np.float32), np.asarray(mb0, np.float32)])[None, :]
    com["mw0r"] = np.tile(np.asarray(mw0[256], np.float32), (P, 1))
    com["mw1b"] = np.concatenate(
        [np.asarray(mw1, np.float32), np.asarray(mb1, np.float32)[None, :]], axis=0)
    com["mw2b"] = np.concatenate(
        [np.asarray(mw2, np.float32),
         np.asarray(mb2, np.float32).reshape(1, 1)], axis=0)

    nc = _build_nc(em, cm)
    in_maps = []
    for k in range(NCORE):
        m = dict(com)
        m["xT"] = xpad[k * NSH : (k + 1) * NSH].T.copy()
        m["gidx"] = edata[k]["gidx"]
        m["dstloc"] = edata[k]["dstloc"]
        m["val"] = edata[k]["val"]
        m["cu"] = cdata[k]["cu"]
        m["cv"] = cdata[k]["cv"]
        m["feat"] = cdata[k]["feat"]
        m["maskr"] = cdata[k]["mask"]
        in_maps.append(m)
    import os
    trace = bool(os.environ.get("KERNEL_TRACE"))
    if trace:
        import types
        import ctypes
        if "antenv.axon_hooks" not in sys.modules:
            try:
                import antenv
                from trn_agent_boot.trn_boot import _ntff_profile_via_ctypes
                mod = types.ModuleType("antenv.axon_hooks")
                hook = [_ntff_profile_via_ctypes("/opt/axon/libaxon_pjrt.so")]
                mod.set_axon_ntff_profile_hook = lambda h: hook.__setitem__(0, h)
                mod.get_axon_ntff_profile_hook = lambda: hook[0]
                sys.modules["antenv.axon_hooks"] = mod
                antenv.axon_hooks = mod
            except Exception:
                trace = False
    res = run_bass_kernel_spmd(nc, in_maps, core_ids=list(range(NCORE)),
                               trace=trace,
                               tmpdir=os.environ.get("KERNEL_TRACE_DIR"))
    if trace and res.exec_time_ns is not None:
        print(f"HW exec time: {res.exec_time_ns} ns")
    y_all = np.zeros(C, np.float32)
    p_all = np.zeros(C, np.float32)
    ncc = cm["ncc"]
    p_lin = res.results[0]["p_out"].ravel()   # global order: k, p, c
    for k in range(NCORE):
        sm = cdata[k]["slotmap"]
        valid = sm >= 0
        j = np.nonzero(valid)[0]              # slot j = c*128 + p
        yk = res.results[k]["y_out"]          # [128, NCC] -> value at [j%128, j//128]
        y_all[sm[valid]] = yk[j % P, j // P]
        gs = k * cm["cslot"] + (j % P) * ncc + (j // P)
        p_all[sm[valid]] = p_lin[gs]
    return y_all[:, None], p_all[:, None]


# revision 13
# speedup vs baseline: 1.2255x; 1.2255x over previous
"""Trainium2 Bass kernel for nn_PolicyNetwork3 (2-layer GraphSAGE + edge-MLP).

Design (8 NeuronCores, SPMD single NEFF):
- dst-sharded aggregation: core k owns node block [6272k, 6272k+6272).
- Node tables stored PIECE-MAJOR in 4 tables (rows grouped by 4 row-ranges
  of every core's shard) so the inter-layer AllGather streams piece-wise,
  overlapped with the window loop, and gather indices fit int16.
- Edges bucketed by (128-dst window, src-piece); per-bucket dma_gather
  calls with trailing -1 pads skipped via a per-call valid-count register.
- segment-sum per 128-dst window via one-hot matmuls accumulating in PSUM
  (one-hot weighted by 1/max(deg,1), built on DVE).
- Linear+BN (folded) fused per window, flipped to produce [feat, node];
  layer 1 fuses the candidate-MLP first-layer projection (gq tables).
- Candidate MLP runs in 512-candidate tiles pipelined with the u/v
  gathers; global softmax on-device after an AllGather of y.
"""

import sys

sys.path.insert(0, "/opt/trn_rl_repo")
sys.path.insert(0, "/root/.axon_site")

import numpy as np

import concourse.bacc as bacc
import concourse.bass as bass
import concourse.bass_isa as bass_isa
import concourse.mybir as mybir
import concourse.tile as tile
from concourse import library_config
from concourse.bass_utils import run_bass_kernel_spmd

P = 128
N, E, C = 50000, 800000, 100000
D = 128
NCORE = 8
NSH = 6272          # nodes per core shard (49 blocks of 128)
NTOT = NSH * NCORE  # 50176 padded node table
NWIN = NSH // P     # 49 windows (= 128-node blocks)
CSH = C // NCORE    # 12500 candidates per core
GCALL = 2048        # max idxs per dma_gather call
TGRP = 4            # candidate chunks per MLP tile (512 cands)
NPIECE = 4
PIECE_BLK = [13, 12, 12, 12]          # blocks per piece
PIECE_R0 = [0, 1664, 3200, 4736]      # first row of each piece
PIECE_ROWS = [1664, 1536, 1536, 1536]
BN_EPS = 1e-5
SLOPE = 0.01
F32 = mybir.dt.float32
I32 = mybir.dt.int32
I16 = mybir.dt.int16
AF = mybir.ActivationFunctionType
ALU = mybir.AluOpType


def _wrap16(idx_lin):
    """[n] -> [128, n/16] int16 in the dma_gather wrapped+replicated layout."""
    n = idx_lin.shape[0]
    assert n % 16 == 0
    w = idx_lin.reshape(n // 16, 16).T.astype(np.int16)
    return np.tile(w, (8, 1)).copy()


def gidx_to_cols(arr):
    """[nslot] -> [128, nchunk] with slot i at [i%128, i//128]."""
    n = arr.shape[0]
    return arr.reshape(n // P, P).T.copy()


def _piece_idx(src):
    """global node id -> (piece, idx16 within piece table)."""
    k = src // NSH
    r = src % NSH
    p = np.searchsorted(np.asarray(PIECE_R0[1:]), r, side="right")
    rows = np.asarray(PIECE_ROWS)[p]
    r0 = np.asarray(PIECE_R0)[p]
    return p, k * rows + (r - r0)


def _prep_edges(src, dst, invdeg):
    """Per-(window, src-piece) runs of 128-edge chunks, uniform across cores."""
    core = np.minimum(dst // NSH, NCORE - 1)
    winl = (dst - core * NSH) // P
    sp, sidx = _piece_idx(src)
    key = (core * NWIN + winl) * NPIECE + sp
    nbkt = NCORE * NWIN * NPIECE
    order = np.argsort(key, kind="stable")
    cnt = np.bincount(key, minlength=nbkt).reshape(NCORE, NWIN * NPIECE)
    nch_u = (-(-cnt // P)).max(axis=0)       # [NWIN*NPIECE] uniform chunk counts
    runs = [(w, p) for w in range(NWIN) for p in range(NPIECE)]
    run_nch = [int(nch_u[w * NPIECE + p]) for (w, p) in runs]
    run_off = np.zeros(len(runs) + 1, np.int64)
    np.cumsum(run_nch, out=run_off[1:])
    tot_ch = int(run_off[-1])
    nslot = tot_ch * P

    gidx = np.full((NCORE, nslot), -1, np.int16)     # trailing pads trimmed
    dstloc = np.full((NCORE, nslot), -5.0, np.float32)
    val = np.zeros((NCORE, nslot), np.float32)
    bstart = np.zeros(nbkt + 1, np.int64)
    np.cumsum(np.bincount(key, minlength=nbkt), out=bstart[1:])
    nreal = np.zeros((NCORE, len(runs)), np.int64)
    for k in range(NCORE):
        for ri, (w, p) in enumerate(runs):
            b = (k * NWIN + w) * NPIECE + p
            e0, e1 = bstart[b], bstart[b + 1]
            n = e1 - e0
            nreal[k, ri] = n
            if n == 0:
                continue
            sl = order[e0:e1]
            sl = sl[np.argsort(sidx[sl], kind="stable")]
            pos = int(run_off[ri]) * P
            gidx[k, pos : pos + n] = sidx[sl].astype(np.int16)
            dstloc[k, pos : pos + n] = (dst[sl] % P).astype(np.float32)
            val[k, pos : pos + n] = invdeg[dst[sl]]
    # per-run gather calls (slot_start, n_idx, piece); <= GCALL idx each
    run_calls = []
    for ri, (w, p) in enumerate(runs):
        p0, p1 = int(run_off[ri]) * P, int(run_off[ri + 1]) * P
        calls = []
        q = p0
        while q < p1:
            n = min(GCALL, p1 - q)
            calls.append((q, n, p))
            q += n
        run_calls.append(calls)
    # per-core per-call valid counts
    cnts = []
    for ri in range(len(runs)):
        p0 = int(run_off[ri]) * P
        for (q, n, p) in run_calls[ri]:
            cnts.append([int(np.clip(nreal[k, ri] - (q - p0), 0, n)) for k in range(NCORE)])
    meta = dict(runs=runs, run_nch=run_nch, run_off=run_off, tot_ch=tot_ch,
                nslot=nslot, run_calls=run_calls, ncalls=len(cnts))
    data = [dict(gidx=_wrap16(gidx[k]),
                 dstloc=gidx_to_cols(dstloc[k]),
                 val=gidx_to_cols(val[k]),
                 cnt=np.array([c[k] for c in cnts], np.int32)[None, :]) for k in range(NCORE)]
    return meta, data


def _prep_cands(cand_u, cand_v, cand_feat):
    """Shard candidates, group by (u_piece, v_piece), pad to uniform chunks."""
    up, uidx = _piece_idx(cand_u)
    vp, vidx = _piece_idx(cand_v)
    gsel = up * NPIECE + vp                  # 16 groups
    NG = NPIECE * NPIECE
    percore = [np.arange(k * CSH, (k + 1) * CSH) for k in range(NCORE)]
    groups = [[None] * NG for _ in range(NCORE)]
    for k in range(NCORE):
        ids = percore[k]
        g = gsel[ids]
        o = np.argsort(g, kind="stable")
        ids = ids[o]
        gs = g[o]
        for gi in range(NG):
            gids = ids[gs == gi]
            groups[k][gi] = gids[np.argsort(uidx[gids], kind="stable")]
    gch = np.zeros((NCORE, NG), np.int64)
    for k in range(NCORE):
        for gi in range(NG):
            gch[k, gi] = -(-len(groups[k][gi]) // P)
    gch_u = gch.max(axis=0)
    ncc = int(gch_u.sum())
    cslot = ncc * P
    cu = np.full((NCORE, cslot), -1, np.int16)
    cv = np.full((NCORE, cslot), -1, np.int16)
    ft = np.zeros((NCORE, cslot), np.float32)
    mask = np.full((NCORE, cslot), -1e30, np.float32)
    slotmap = np.full((NCORE, cslot), -1, np.int64)
    nreal = np.zeros((NCORE, NG), np.int64)
    goff = np.zeros(NG + 1, np.int64)
    np.cumsum(gch_u * P, out=goff[1:])
    for k in range(NCORE):
        for gi in range(NG):
            ids = groups[k][gi]
            n = len(ids)
            nreal[k, gi] = n
            p0 = goff[gi]
            cu[k, p0 : p0 + n] = uidx[ids].astype(np.int16)
            cv[k, p0 : p0 + n] = vidx[ids].astype(np.int16)
            ft[k, p0 : p0 + n] = cand_feat[ids, 0]
            mask[k, p0 : p0 + n] = 0.0
            slotmap[k, p0 : p0 + n] = ids
    # u calls: per u-piece (4 consecutive groups); v calls: per group
    ucalls, vcalls = [], []
    for pu in range(NPIECE):
        lo, hi = goff[pu * NPIECE], goff[pu * NPIECE + NPIECE]
        q = lo
        while q < hi:
            n = min(GCALL, hi - q)
            ucalls.append((int(q), int(n), pu))
            q += n
    for gi in range(NG):
        q, hi = goff[gi], goff[gi + 1]
        while q < hi:
            n = min(GCALL, hi - q)
            vcalls.append((int(q), int(n), gi % NPIECE))
            q += n
    meta = dict(ncc=ncc, cslot=cslot, ucalls=ucalls, vcalls=vcalls,
                goff=goff, nreal=nreal)
    data = [dict(cu=cu[k], cv=cv[k],
                 feat=gidx_to_cols(ft[k]), mask=gidx_to_cols(mask[k]),
                 slotmap=slotmap[k]) for k in range(NCORE)]
    return meta, data


def _finish_cands(cm, cdata):
    """Fix pad validity: within each call, pads before the last real slot
    must be valid indices (0); only trailing pads stay -1. Compute per-call
    valid counts = index of last real slot + 1 (trim point)."""
    NG = NPIECE * NPIECE
    goff, nreal = cm["goff"], cm["nreal"]
    ucnt = [[0] * NCORE for _ in cm["ucalls"]]
    vcnt = [[0] * NCORE for _ in cm["vcalls"]]
    for k in range(NCORE):
        cu, cv = cdata[k]["cu"], cdata[k]["cv"]
        real = np.zeros(cm["cslot"], bool)
        for gi in range(NG):
            real[goff[gi] : goff[gi] + nreal[k, gi]] = True
        for calls, cnts, arr in ((cm["ucalls"], ucnt, cu), (cm["vcalls"], vcnt, cv)):
            for ci, (q, n, _s) in enumerate(calls):
                r = np.nonzero(real[q : q + n])[0]
                last = int(r[-1]) + 1 if len(r) else 0
                cnts[ci][k] = last
                # interior pads -> valid row 0
                pad = ~real[q : q + last]
                if pad.any():
                    arr[q : q + last][pad] = 0
    for k in range(NCORE):
        cdata[k]["ccnt"] = np.array(
            [c[k] for c in ucnt] + [c[k] for c in vcnt], np.int32)[None, :]
        cdata[k]["cu"] = _wrap16(cdata[k]["cu"])
        cdata[k]["cv"] = _wrap16(cdata[k]["cv"])
    cm["nccalls"] = len(ucnt) + len(vcnt)


def _build_nc(em, cm):
    nc = bacc.Bacc("TRN2", target_bir_lowering=False, debug=False,
                   num_devices=NCORE)
    f32 = F32
    TOTCH, NSLOT = em["tot_ch"], em["nslot"]
    NCC, CSLOT = cm["ncc"], cm["cslot"]
    NCALL, NCCALL = em["ncalls"], cm["nccalls"]

    # ---- external inputs ----
    xp = [nc.dram_tensor(f"xp{p}", [NCORE * PIECE_ROWS[p], D], f32, kind="ExternalInput")
          for p in range(NPIECE)]
    xT = nc.dram_tensor("xT", [P, NSH], f32, kind="ExternalInput")
    gidx = nc.dram_tensor("gidx", [P, NSLOT // 16], I16, kind="ExternalInput")
    dstloc = nc.dram_tensor("dstloc", [P, TOTCH], f32, kind="ExternalInput")
    val = nc.dram_tensor("val", [P, TOTCH], f32, kind="ExternalInput")
    cnt = nc.dram_tensor("cnt", [1, NCALL], I32, kind="ExternalInput")
    ccnt = nc.dram_tensor("ccnt", [1, NCCALL], I32, kind="ExternalInput")
    wself = [nc.dram_tensor(f"wself{l}", [D, D], f32, kind="ExternalInput") for l in range(2)]
    wneigh = [nc.dram_tensor(f"wneigh{l}", [D, D], f32, kind="ExternalInput") for l in range(2)]
    crow = [nc.dram_tensor(f"crow{l}", [1, D], f32, kind="ExternalInput") for l in range(2)]
    iota = nc.dram_tensor("iota", [P, P], f32, kind="ExternalInput")
    ident = nc.dram_tensor("ident", [P, P], f32, kind="ExternalInput")
    onesr = nc.dram_tensor("onesr", [1, P], f32, kind="ExternalInput")
    abmat = nc.dram_tensor("abmat", [D, D], f32, kind="ExternalInput")
    gqbias = nc.dram_tensor("gqbias", [1, D], f32, kind="ExternalInput")
    mw0r = nc.dram_tensor("mw0r", [P, 64], f32, kind="ExternalInput")
    mw1b = nc.dram_tensor("mw1b", [65, 64], f32, kind="ExternalInput")
    mw2b = nc.dram_tensor("mw2b", [65, 1], f32, kind="ExternalInput")
    cu = nc.dram_tensor("cu", [P, CSLOT // 16], I16, kind="ExternalInput")
    cv = nc.dram_tensor("cv", [P, CSLOT // 16], I16, kind="ExternalInput")
    feat = nc.dram_tensor("feat", [P, NCC], f32, kind="ExternalInput")
    maskr = nc.dram_tensor("maskr", [P, NCC], f32, kind="ExternalInput")
    # ---- outputs ----
    y_out = nc.dram_tensor("y_out", [P, NCC], f32, kind="ExternalOutput")
    p_out = nc.dram_tensor("p_out", [P, NCORE * CSLOT // P], f32, kind="ExternalOutput")
    # ---- internal DRAM ----
    hshp = [nc.dram_tensor(f"hshp{p}", [PIECE_ROWS[p], D], f32, kind="Internal")
            for p in range(NPIECE)]
    hfullp = [nc.dram_tensor(f"hfullp{p}", [NCORE * PIECE_ROWS[p], D], f32,
                             kind="Internal", addr_space="Shared") for p in range(NPIECE)]
    gqshp = [nc.dram_tensor(f"gqshp{p}", [PIECE_ROWS[p], D], f32, kind="Internal")
             for p in range(NPIECE)]
    gqfullp = [nc.dram_tensor(f"gqfullp{p}", [NCORE * PIECE_ROWS[p], D], f32,
                              kind="Internal", addr_space="Shared") for p in range(NPIECE)]
    ysh = nc.dram_tensor("ysh", [P, NCC], f32, kind="Internal")
    yfull = nc.dram_tensor("yfull", [NCORE * P, NCC], f32, kind="Internal", addr_space="Shared")

    rg = [list(range(NCORE))]
    runs, run_nch, run_calls = em["runs"], em["run_nch"], em["run_calls"]
    # piece boundary: after which window is each piece's shard complete
    pend = np.cumsum(PIECE_BLK) - 1          # windows 12, 24, 36, 48

    with tile.TileContext(nc) as tc:
        with (
            tc.tile_pool(name="const", bufs=1) as cp,
            tc.tile_pool(name="big", bufs=1) as bp,
            tc.tile_pool(name="msgs", bufs=3) as mp,
            tc.tile_pool(name="oh", bufs=4) as ohp,
            tc.tile_pool(name="wrk", bufs=4) as wp,
            tc.tile_pool(name="zt", bufs=3) as zp,
            tc.tile_pool(name="ps_run", bufs=2, space="PSUM") as ps_run,
            tc.tile_pool(name="ps_t", bufs=2, space="PSUM") as ps_t,
            tc.tile_pool(name="ps_h", bufs=2, space="PSUM") as ps_h,
        ):
            nc.gpsimd.load_library(library_config.mlp)

            def load(pool, t, shape=None):
                tl = pool.tile(shape or list(t.shape), t.dtype, tag=t.name)
                nc.sync.dma_start(tl[:], t[:])
                return tl

            gidx_t = load(cp, gidx)
            dstloc_t = load(cp, dstloc)
            val_t = load(cp, val)
            cnt_t = load(cp, cnt)
            ccnt_t = load(cp, ccnt)
            iota_t = load(cp, iota)
            ident_t = load(cp, ident)
            onesr_t = load(cp, onesr)
            wself_t = [load(cp, w) for w in wself]
            wneigh_t = [load(cp, w) for w in wneigh]
            crow_t = [load(cp, w) for w in crow]
            abmat_t = load(cp, abmat)
            gqbias_t = load(cp, gqbias)
            mw0r_t = load(cp, mw0r)
            mw1b_t = load(cp, mw1b)
            mw2b_t = load(cp, mw2b)
            cu_t = load(cp, cu)
            cv_t = load(cp, cv)
            feat_t = load(cp, feat)
            mask_t = load(cp, maskr)

            curT = bp.tile([P, NSH], f32, tag="curT")
            nxtT = bp.tile([P, NSH], f32, tag="nxtT")
            nc.sync.dma_start(curT[:], xT[:])

            creg = nc.gpsimd.alloc_register("gcnt")

            # zero gather bufs once: short calls leave stale tail chunks
            # whose one-hot is all-zero; 0*NaN would poison the psum
            for _ in range(3):
                g0 = mp.tile([P, GCALL // P, P], f32, tag="g")
                nc.vector.memset(g0[:], 0.0)
            ut = bp.tile([P, NCC, 64], f32, tag="ut")
            vt = bp.tile([P, NCC, 64], f32, tag="vt")
            nc.vector.memset(ut[:], 0.0)
            nc.vector.memset(vt[:], 0.0)

            for layer in range(2):
                tabs = xp if layer == 0 else hfullp
                ci = 0
                for w in range(NWIN):
                    wch = sum(run_nch[w * NPIECE + p] for p in range(NPIECE))
                    aggw = wp.tile([P, P], f32, tag="aggw")
                    if wch == 0:
                        nc.vector.memset(aggw[:], 0.0)
                    else:
                        ps = ps_run.tile([P, P], f32, tag="psw")
                        jw = 0
                        for p in range(NPIECE):
                            for (s0, n_idx, pc) in run_calls[w * NPIECE + p]:
                                ncall = n_idx // P
                                nc.gpsimd.reg_load(creg, cnt_t[0:1, ci : ci + 1])
                                ci += 1
                                g = mp.tile([P, GCALL // P, P], f32, tag="g")
                                nc.gpsimd.dma_gather(
                                    g[:, :ncall, :], tabs[pc][:],
                                    gidx_t[:, s0 // 16 : (s0 + n_idx) // 16],
                                    n_idx, creg, P, single_packet=False)
                                for cc in range(ncall):
                                    ch = s0 // P + cc
                                    oh = ohp.tile([P, P], f32, tag="oh")
                                    nc.vector.tensor_scalar(
                                        oh[:], iota_t[:], dstloc_t[:, ch : ch + 1],
                                        val_t[:, ch : ch + 1], ALU.is_equal, ALU.mult)
                                    nc.tensor.matmul(ps[:], lhsT=g[:, cc, :], rhs=oh[:],
                                                     start=(jw == 0), stop=(jw == wch - 1))
                                    jw += 1
                        nc.scalar.activation(aggw[:], ps[:], AF.Copy)
                    nb = w * P
                    ph = ps_h.tile([P, P], f32, tag="ph")
                    nc.tensor.matmul(ph[:], lhsT=wneigh_t[layer][:], rhs=aggw[:],
                                     start=True, stop=False)
                    nc.tensor.matmul(ph[:], lhsT=wself_t[layer][:],
                                     rhs=curT[:, nb : nb + P], start=False, stop=False)
                    nc.tensor.matmul(ph[:], lhsT=crow_t[layer][:], rhs=onesr_t[:],
                                     start=False, stop=True)
                    tmp = wp.tile([P, P], f32, tag="tmp")
                    nc.vector.tensor_scalar(tmp[:], ph[:], SLOPE, None, ALU.mult)
                    nc.vector.tensor_tensor(nxtT[:, nb : nb + P], ph[:], tmp[:], ALU.max)
                    wp_, wr0 = next((i, PIECE_R0[i]) for i in range(NPIECE)
                                    if PIECE_R0[i] <= nb < PIECE_R0[i] + PIECE_BLK[i] * P)
                    rb = nb - wr0
                    if layer == 0:
                        pt = ps_t.tile([P, P], f32, tag="pt")
                        nc.tensor.transpose(pt[:], nxtT[:, nb : nb + P], ident_t[:])
                        hb = wp.tile([P, P], f32, tag="hb")
                        nc.scalar.activation(hb[:], pt[:], AF.Copy)
                        nc.sync.dma_start(hshp[wp_][rb : rb + P, :], hb[:])
                    else:
                        pg = ps_h.tile([P, P], f32, tag="ph")
                        nc.tensor.matmul(pg[:], lhsT=nxtT[:, nb : nb + P],
                                         rhs=abmat_t[:], start=True, stop=False)
                        nc.tensor.matmul(pg[:], lhsT=onesr_t[:], rhs=gqbias_t[:],
                                         start=False, stop=True)
                        gb = wp.tile([P, P], f32, tag="gb")
                        nc.scalar.activation(gb[:], pg[:], AF.Copy)
                        nc.sync.dma_start(gqshp[wp_][rb : rb + P, :], gb[:])
                    if w in pend:
                        pi = int(np.nonzero(pend == w)[0][0])
                        if layer == 0:
                            nc.gpsimd.collective_compute(
                                "AllGather", ALU.bypass, replica_groups=rg,
                                ins=[hshp[pi][:].opt()], outs=[hfullp[pi][:].opt()])
                        else:
                            nc.gpsimd.collective_compute(
                                "AllGather", ALU.bypass, replica_groups=rg,
                                ins=[gqshp[pi][:].opt()], outs=[gqfullp[pi][:].opt()])
                if layer == 0:
                    curT, nxtT = nxtT, curT

            # ---- candidate gathers (u then v, interleaved by slot) ----
            merged = sorted(
                [(s0, n, s, 0, ci) for ci, (s0, n, s) in enumerate(cm["ucalls"])]
                + [(s0, n, s, 1, len(cm["ucalls"]) + ci)
                   for ci, (s0, n, s) in enumerate(cm["vcalls"])],
                key=lambda t: (t[0], t[3]))
            for (s0, n_idx, pc, which, ci) in merged:
                tl, idx_t, cofs = ((ut, cu_t, 0), (vt, cv_t, 64))[which]
                nc.gpsimd.reg_load(creg, ccnt_t[0:1, ci : ci + 1])
                nc.gpsimd.dma_gather(
                    tl[:, s0 // P : (s0 + n_idx) // P, :],
                    gqfullp[pc][:, cofs : cofs + 64],
                    idx_t[:, s0 // 16 : (s0 + n_idx) // 16],
                    n_idx, creg, 64, elem_step=P, single_packet=False)

            # ---- candidate MLP in 512-cand tiles ----
            ycol = bp.tile([P, NCC], f32, tag="ycol")
            for t0 in range(0, NCC, TGRP):
                tn = min(TGRP, NCC - t0)
                z1 = zp.tile([P, TGRP, 64], f32, tag="z1")
                for c in range(tn):
                    nc.vector.tensor_scalar(z1[:, c, :], mw0r_t[:],
                                            feat_t[:, t0 + c : t0 + c + 1],
                                            None, ALU.mult)
                nc.vector.tensor_tensor(z1[:, :tn, :], z1[:, :tn, :],
                                        ut[:, t0 : t0 + tn, :], ALU.add)
                nc.vector.tensor_tensor(z1[:, :tn, :], z1[:, :tn, :],
                                        vt[:, t0 : t0 + tn, :], ALU.add)
                zs = zp.tile([P, TGRP, 64], f32, tag="zs")
                nc.vector.tensor_scalar(zs[:, :tn, :], z1[:, :tn, :], SLOPE,
                                        None, ALU.mult)
                nc.vector.tensor_tensor(z1[:, :tn, :], z1[:, :tn, :],
                                        zs[:, :tn, :], ALU.max)
                z1t = zp.tile([65, TGRP * P], f32, tag="z1t")
                nc.vector.memset(z1t[64:65, :], 1.0)
                for c in range(tn):
                    pt2 = ps_t.tile([64, P], f32, tag="pt")
                    nc.tensor.transpose(pt2[:], z1[:, c, :], ident_t[:])
                    nc.scalar.activation(z1t[0:64, c * P : (c + 1) * P], pt2[:], AF.Copy)
                ps2 = ps_run.tile([64, TGRP * P], f32, tag="ps2")
                nc.tensor.matmul(ps2[:, : tn * P], lhsT=mw1b_t[:],
                                 rhs=z1t[:, : tn * P], start=True, stop=True)
                z2t = zp.tile([65, TGRP * P], f32, tag="z2t")
                nc.vector.memset(z2t[64:65, :], 1.0)
                nc.vector.tensor_scalar(z2t[0:64, : tn * P], ps2[:, : tn * P],
                                        SLOPE, None, ALU.mult)
                nc.vector.tensor_tensor(z2t[0:64, : tn * P], z2t[0:64, : tn * P],
                                        ps2[:, : tn * P], ALU.max)
                for c in range(tn):
                    py = ps_h.tile([P, 1], f32, tag="ph")
                    nc.tensor.matmul(py[:], lhsT=z2t[:, c * P : (c + 1) * P],
                                     rhs=mw2b_t[:], start=True, stop=True)
                    nc.scalar.activation(ycol[:, t0 + c : t0 + c + 1], py[:], AF.Copy)

            nc.sync.dma_start(y_out[:], ycol[:])
            ym = wp.tile([P, NCC], f32, tag="ym")
            nc.vector.tensor_tensor(ym[:], ycol[:], mask_t[:], ALU.add)
            nc.sync.dma_start(ysh[:], ym[:])
            nc.gpsimd.collective_compute(
                "AllGather", ALU.bypass, replica_groups=rg,
                ins=[ysh[:].opt()], outs=[yfull[:].opt()])
            # ---- softmax ----
            ncols = NCORE * CSLOT // P
            yf = bp.tile([P, ncols], f32, tag="yf")
            nc.sync.dma_start(yf[:], yfull[:].rearrange("a b -> (a b)")
                              .rearrange("(p c) -> p c", p=P))
            rmax = wp.tile([P, 1], f32, tag="rmax")
            nc.vector.tensor_reduce(rmax[:], yf[:], mybir.AxisListType.X, ALU.max)
            gmax = wp.tile([P, 1], f32, tag="gmax")
            nc.gpsimd.partition_all_reduce(gmax[:], rmax[:], P,
                                           bass_isa.ReduceOp.max)
            ngmax = wp.tile([P, 1], f32, tag="ngmax")
            nc.vector.tensor_scalar(ngmax[:], gmax[:], -1.0, None, ALU.mult)
            ef = bp.tile([P, ncols], f32, tag="ef")
            se = wp.tile([P, 1], f32, tag="se")
            nc.scalar.activation(ef[:], yf[:], AF.Exp, bias=ngmax[:, 0:1],
                                 accum_out=se[:])
            stot = wp.tile([P, 1], f32, tag="stot")
            nc.gpsimd.partition_all_reduce(stot[:], se[:], P, bass_isa.ReduceOp.add)
            invs = wp.tile([P, 1], f32, tag="invs")
            nc.vector.reciprocal(invs[:], stot[:])
            pf = bp.tile([P, ncols], f32, tag="pf")
            nc.vector.tensor_scalar(pf[:], ef[:], invs[:, 0:1], None, ALU.mult)
            nc.sync.dma_start(p_out[:], pf[:])
    nc.compile()
    return nc


def kernel(x, src, dst, cand_u, cand_v, cand_feat,
           w_self0, w_neigh0, b0, gamma0, beta0, rm0, rv0,
           w_self1, w_neigh1, b1, gamma1, beta1, rm1, rv1,
           mw0, mb0, mw1, mb1, mw2, mb2):
    x = np.asarray(x, np.float32)
    src = np.asarray(src, np.int64)
    dst = np.asarray(dst, np.int64)
    cand_u = np.asarray(cand_u, np.int64)
    cand_v = np.asarray(cand_v, np.int64)
    cand_feat = np.asarray(cand_feat, np.float32)

    deg = np.bincount(dst, minlength=N).astype(np.float32)
    invdeg = 1.0 / np.maximum(deg, 1.0)
    em, edata = _prep_edges(src, dst, invdeg)
    cm, cdata = _prep_cands(cand_u, cand_v, cand_feat)
    _finish_cands(cm, cdata)

    xpad = np.zeros((NTOT, D), np.float32)
    xpad[:N] = x
    iota = np.tile(np.arange(P, dtype=np.float32), (P, 1))
    ident = np.eye(P, dtype=np.float32)
    onesr = np.ones((1, P), np.float32)

    com = {"iota": iota, "ident": ident, "onesr": onesr}
    # piece-major x tables
    xsh = xpad.reshape(NCORE, NSH, D)
    for p in range(NPIECE):
        r0, rows = PIECE_R0[p], PIECE_ROWS[p]
        com[f"xp{p}"] = xsh[:, r0 : r0 + rows, :].reshape(NCORE * rows, D).copy()
    for l, (ws, wn, b, ga, be, rme, rve) in enumerate(
        ((w_self0, w_neigh0, b0, gamma0, beta0, rm0, rv0),
         (w_self1, w_neigh1, b1, gamma1, beta1, rm1, rv1))):
        a = (ga / np.sqrt(rve + BN_EPS)).astype(np.float32)
        com[f"wself{l}"] = (ws * a[None, :]).astype(np.float32)
        com[f"wneigh{l}"] = (wn * a[None, :]).astype(np.float32)
        com[f"crow{l}"] = (a * (b - rme) + be).astype(np.float32)[None, :]
    com["abmat"] = np.concatenate(
        [np.asarray(mw0[0:128], np.float32), np.asarray(mw0[128:256], np.float32)], axis=1)
    com["gqbias"] = np.concatenate(
        [np.zeros(64, np.float32), np.asarray(mb0, np.float32)])[None, :]
    com["mw0r"] = np.tile(np.asarray(mw0[256], np.float32), (P, 1))
    com["mw1b"] = np.concatenate(
        [np.asarray(mw1, np.float32), np.asarray(mb1, np.float32)[None, :]], axis=0)
    com["mw2b"] = np.concatenate(
        [np.asarray(mw2, np.float32),
         np.asarray(mb2, np.float32).reshape(1, 1)], axis=0)

    nc = _build_nc(em, cm)
    in_maps = []
    for k in range(NCORE):
        m = dict(com)
        m["xT"] = xpad[k * NSH : (k + 1) * NSH].T.copy()
        m["gidx"] = edata[k]["gidx"]
        m["dstloc"] = edata[k]["dstloc"]
        m["val"] = edata[k]["val"]
        m["cnt"] = edata[k]["cnt"]
        m["ccnt"] = cdata[k]["ccnt"]
        m["cu"] = cdata[k]["cu"]
        m["cv"] = cdata[k]["cv"]
        m["feat"] = cdata[k]["feat"]
        m["maskr"] = cdata[k]["mask"]
        in_maps.append(m)
    import os
    trace = bool(os.environ.get("KERNEL_TRACE"))
    if trace:
        import types
        import ctypes
        if "antenv.axon_hooks" not in sys.modules:
            try:
                import antenv
                from trn_agent_boot.trn_boot import _ntff_profile_via_ctypes
                mod = types.ModuleType("antenv.axon_hooks")
                hook = [_ntff_profile_via_ctypes("/opt/axon/libaxon_pjrt.so")]
                mod.set_axon_ntff_profile_hook = lambda h: hook.__setitem__(0, h)
                mod.get_axon_ntff_profile_hook = lambda: hook[0]
                sys.modules["antenv.axon_hooks"] = mod
                antenv.axon_hooks = mod
            except Exception:
                trace = False
    res = run_bass_kernel_spmd(nc, in_maps, core_ids=list(range(NCORE)),
                               trace=trace,
                               tmpdir=os.environ.get("KERNEL_TRACE_DIR"))
    if trace and res.exec_time_ns is not None:
        print(f"HW exec time: {res.exec_time_ns} ns")
    y_all = np.zeros(C, np.float32)
    p_all = np.zeros(C, np.float32)
    ncc = cm["ncc"]
    p_lin = res.results[0]["p_out"].ravel()   # global order: k, p, c
    for k in range(NCORE):
        sm = cdata[k]["slotmap"]
        valid = sm >= 0
        j = np.nonzero(valid)[0]              # slot j = c*128 + p
        yk = res.results[k]["y_out"]          # [128, NCC] -> value at [j%128, j//128]
        y_all[sm[valid]] = yk[j % P, j // P]
        gs = k * cm["cslot"] + (j % P) * ncc + (j // P)
        p_all[sm[valid]] = p_lin[gs]
    return y_all[:, None], p_all[:, None]


# revision 14
# speedup vs baseline: 1.3528x; 1.1039x over previous
"""Trainium2 Bass kernel for nn_PolicyNetwork3 (2-layer GraphSAGE + edge-MLP).

Design (8 NeuronCores, SPMD single NEFF):
- dst-sharded aggregation: core k owns node block [6272k, 6272k+6272).
- Node tables stored PIECE-MAJOR in one concatenated [50176, 128] table;
  the inter-layer AllGather streams piece-wise into row slices of it,
  overlapped with the window loop. Gather indices are SIGNED int16 offsets
  from the table midpoint (the Q7 address math sign-extends), so one call
  covers the whole table.
- Edges bucketed per 128-dst window; ONE dma_gather call per window
  (amortizes the ~3us per-call Q7 overhead). Every call's last index is
  kept non-negative so the ucode's trailing-negative trim never fires.
- segment-sum per window via one-hot matmuls accumulating in PSUM
  (one-hot weighted by 1/max(deg,1), built on DVE).
- Linear+BN (folded) fused per window, flipped to produce [feat, node];
  layer 1 fuses the candidate-MLP first-layer projection (gq tables).
- Candidate MLP runs in 512-candidate tiles pipelined with the u/v
  gathers; global softmax on-device after an AllGather of y.
"""

import sys

sys.path.insert(0, "/opt/trn_rl_repo")
sys.path.insert(0, "/root/.axon_site")

import numpy as np

import concourse.bacc as bacc
import concourse.bass as bass
import concourse.bass_isa as bass_isa
import concourse.mybir as mybir
import concourse.tile as tile
from concourse import library_config
from concourse.bass_utils import run_bass_kernel_spmd

P = 128
N, E, C = 50000, 800000, 100000
D = 128
NCORE = 8
NSH = 6272          # nodes per core shard (49 blocks of 128)
NTOT = NSH * NCORE  # 50176 padded node table
NWIN = NSH // P     # 49 windows (= 128-node blocks)
HALF = NTOT // 2    # gather base row (signed idx16 offsets from here)
CSH = C // NCORE    # 12500 candidates per core
GMAX = 2560         # max idxs per dma_gather call
TGRP = 4            # candidate chunks per MLP tile (512 cands)
NPIECE = 4
PIECE_BLK = [13, 12, 12, 12]          # blocks per piece
PIECE_R0 = [0, 1664, 3200, 4736]      # first local row of each piece
PIECE_ROWS = [1664, 1536, 1536, 1536]
PIECE_CATB = [0, 13312, 25600, 37888]  # piece base row in the cat table
BN_EPS = 1e-5
SLOPE = 0.01
F32 = mybir.dt.float32
I16 = mybir.dt.int16
AF = mybir.ActivationFunctionType
ALU = mybir.AluOpType


def _wrap16(idx_lin):
    """[n] -> [128, n/16] int16 in the dma_gather wrapped+replicated layout."""
    n = idx_lin.shape[0]
    assert n % 16 == 0
    w = idx_lin.reshape(n // 16, 16).T.astype(np.int16)
    return np.tile(w, (8, 1)).copy()


def gidx_to_cols(arr):
    """[nslot] -> [128, nchunk] with slot i at [i%128, i//128]."""
    n = arr.shape[0]
    return arr.reshape(n // P, P).T.copy()


def _cat_row(src):
    """global node id -> row in the piece-major cat table."""
    src = np.asarray(src)
    k = src // NSH
    r = src % NSH
    p = np.searchsorted(np.asarray(PIECE_R0[1:]), r, side="right")
    return (np.asarray(PIECE_CATB)[p] + k * np.asarray(PIECE_ROWS)[p]
            + (r - np.asarray(PIECE_R0)[p]))


def _split_calls(p0, p1):
    """slot range -> near-equal calls of <= GMAX idxs (multiples of 128)."""
    total = p1 - p0
    ncall = -(-total // GMAX)
    per = -(-(total // P) // ncall) * P
    calls = []
    q = p0
    while q < p1:
        n = min(per, p1 - q)
        calls.append((q, n))
        q += n
    return calls


def _guard_calls(idxmats, calls):
    """Ensure the last slot of every call has all idx matrices >= 0 there,
    swapping in a suitable slot from the same call (or whole range)."""
    nmat = len(idxmats)
    ok = np.ones(idxmats[0].shape[0], bool)
    for m in idxmats:
        ok &= (m >= 0)
    lasts = {q + n - 1 for (q, n) in calls}
    for (q, n) in calls:
        last = q + n - 1
        if ok[last]:
            continue
        cand = [j for j in range(q, q + n - 1) if ok[j] and j not in lasts]
        assert cand, "no non-negative slot available for call guard"
        j = cand[-1]
        for m in idxmats:
            m[last], m[j] = m[j], m[last]
    return idxmats


def _prep_edges(src, dst, invdeg):
    """Per-window buckets of 128-edge chunks, uniform across cores."""
    core = np.minimum(dst // NSH, NCORE - 1)
    winl = (dst - core * NSH) // P
    g = _cat_row(src)
    key = core * NWIN + winl
    nbkt = NCORE * NWIN
    order = np.argsort(key, kind="stable")
    cnt = np.bincount(key, minlength=nbkt).reshape(NCORE, NWIN)
    nch_u = (-(-cnt // P)).max(axis=0)       # [NWIN] uniform chunk counts
    run_off = np.zeros(NWIN + 1, np.int64)
    np.cumsum(nch_u, out=run_off[1:])
    tot_ch = int(run_off[-1])
    nslot = tot_ch * P

    gidx = np.zeros((NCORE, nslot), np.int16)        # pads read row HALF
    dstloc = np.full((NCORE, nslot), -5.0, np.float32)
    val = np.zeros((NCORE, nslot), np.float32)
    bstart = np.zeros(nbkt + 1, np.int64)
    np.cumsum(np.bincount(key, minlength=nbkt), out=bstart[1:])
    run_calls = [_split_calls(int(run_off[w]) * P, int(run_off[w + 1]) * P)
                 for w in range(NWIN)]
    for k in range(NCORE):
        for w in range(NWIN):
            b = k * NWIN + w
            e0, e1 = bstart[b], bstart[b + 1]
            n = e1 - e0
            if n == 0:
                continue
            sl = order[e0:e1]
            sl = sl[np.argsort(g[sl], kind="stable")]
            pos = int(run_off[w]) * P
            gidx[k, pos : pos + n] = (g[sl] - HALF).astype(np.int16)
            dstloc[k, pos : pos + n] = (dst[sl] % P).astype(np.float32)
            val[k, pos : pos + n] = invdeg[dst[sl]]
        # keep last slot of every call non-negative (trim guard); swap
        # dstloc/val along with the index
        for w in range(NWIN):
            calls = run_calls[w]
            lasts = [q + n - 1 for (q, n) in calls]
            for (q, n) in calls:
                last = q + n - 1
                if gidx[k, last] >= 0:
                    continue
                p0, p1 = int(run_off[w]) * P, int(run_off[w + 1]) * P
                cand = [j for j in range(p1 - 1, p0 - 1, -1)
                        if gidx[k, j] >= 0 and j not in lasts]
                assert cand, "no non-negative index in window bucket"
                j = cand[0]
                for m in (gidx, dstloc, val):
                    m[k, last], m[k, j] = m[k, j], m[k, last]
    meta = dict(run_off=run_off, run_nch=[int(x) for x in nch_u],
                tot_ch=tot_ch, nslot=nslot, run_calls=run_calls)
    data = [dict(gidx=_wrap16(gidx[k]),
                 dstloc=gidx_to_cols(dstloc[k]),
                 val=gidx_to_cols(val[k])) for k in range(NCORE)]
    return meta, data


def _prep_cands(cand_u, cand_v, cand_feat):
    """Shard candidates per core, sort by u row, pad to uniform chunks."""
    gu = _cat_row(cand_u) - HALF
    gv = _cat_row(cand_v) - HALF
    ncc = -(-CSH // P)
    cslot = ncc * P
    cu = np.zeros((NCORE, cslot), np.int16)
    cv = np.zeros((NCORE, cslot), np.int16)
    ft = np.zeros((NCORE, cslot), np.float32)
    mask = np.full((NCORE, cslot), -1e30, np.float32)
    slotmap = np.full((NCORE, cslot), -1, np.int64)
    calls = _split_calls(0, cslot)
    for k in range(NCORE):
        ids = np.arange(k * CSH, (k + 1) * CSH)
        ids = ids[np.argsort(gu[ids], kind="stable")]
        cu[k, :CSH] = gu[ids].astype(np.int16)
        cv[k, :CSH] = gv[ids].astype(np.int16)
        ft[k, :CSH] = cand_feat[ids, 0]
        mask[k, :CSH] = 0.0
        slotmap[k, :CSH] = ids
        # joint guard: last slot of each call needs cu>=0 AND cv>=0
        both = (cu[k] >= 0) & (cv[k] >= 0)
        lasts = [q + n - 1 for (q, n) in calls]
        for (q, n) in calls:
            last = q + n - 1
            if both[last]:
                continue
            cand = [j for j in range(q + n - 2, q - 1, -1)
                    if both[j] and j not in lasts]
            if not cand:
                cand = [j for j in range(cslot - 1, -1, -1)
                        if both[j] and j not in lasts]
            assert cand, "no jointly non-negative candidate for call guard"
            j = cand[0]
            for m in (cu, cv, ft, mask):
                m[k, last], m[k, j] = m[k, j], m[k, last]
            slotmap[k, last], slotmap[k, j] = slotmap[k, j], slotmap[k, last]
            both[last], both[j] = both[j], both[last]
    meta = dict(ncc=ncc, cslot=cslot, calls=calls)
    data = [dict(cu=_wrap16(cu[k]), cv=_wrap16(cv[k]),
                 feat=gidx_to_cols(ft[k]), mask=gidx_to_cols(mask[k]),
                 slotmap=slotmap[k]) for k in range(NCORE)]
    return meta, data


def _build_nc(em, cm):
    nc = bacc.Bacc("TRN2", target_bir_lowering=False, debug=False,
                   num_devices=NCORE)
    f32 = F32
    TOTCH, NSLOT = em["tot_ch"], em["nslot"]
    NCC, CSLOT = cm["ncc"], cm["cslot"]

    # ---- external inputs ----
    xcat = nc.dram_tensor("xcat", [NTOT, D], f32, kind="ExternalInput")
    xT = nc.dram_tensor("xT", [P, NSH], f32, kind="ExternalInput")
    gidx = nc.dram_tensor("gidx", [P, NSLOT // 16], I16, kind="ExternalInput")
    dstloc = nc.dram_tensor("dstloc", [P, TOTCH], f32, kind="ExternalInput")
    val = nc.dram_tensor("val", [P, TOTCH], f32, kind="ExternalInput")
    wself = [nc.dram_tensor(f"wself{l}", [D, D], f32, kind="ExternalInput") for l in range(2)]
    wneigh = [nc.dram_tensor(f"wneigh{l}", [D, D], f32, kind="ExternalInput") for l in range(2)]
    crow = [nc.dram_tensor(f"crow{l}", [1, D], f32, kind="ExternalInput") for l in range(2)]
    iota = nc.dram_tensor("iota", [P, P], f32, kind="ExternalInput")
    ident = nc.dram_tensor("ident", [P, P], f32, kind="ExternalInput")
    onesr = nc.dram_tensor("onesr", [1, P], f32, kind="ExternalInput")
    abmat = nc.dram_tensor("abmat", [D, D], f32, kind="ExternalInput")
    gqbias = nc.dram_tensor("gqbias", [1, D], f32, kind="ExternalInput")
    mw0r = nc.dram_tensor("mw0r", [P, 64], f32, kind="ExternalInput")
    mw1b = nc.dram_tensor("mw1b", [65, 64], f32, kind="ExternalInput")
    mw2b = nc.dram_tensor("mw2b", [65, 1], f32, kind="ExternalInput")
    cu = nc.dram_tensor("cu", [P, CSLOT // 16], I16, kind="ExternalInput")
    cv = nc.dram_tensor("cv", [P, CSLOT // 16], I16, kind="ExternalInput")
    feat = nc.dram_tensor("feat", [P, NCC], f32, kind="ExternalInput")
    maskr = nc.dram_tensor("maskr", [P, NCC], f32, kind="ExternalInput")
    # ---- outputs ----
    y_out = nc.dram_tensor("y_out", [P, NCC], f32, kind="ExternalOutput")
    p_out = nc.dram_tensor("p_out", [P, NCORE * CSLOT // P], f32, kind="ExternalOutput")
    # ---- internal DRAM ----
    hshp = [nc.dram_tensor(f"hshp{p}", [PIECE_ROWS[p], D], f32, kind="Internal")
            for p in range(NPIECE)]
    hcat = nc.dram_tensor("hcat", [NTOT, D], f32, kind="Internal", addr_space="Shared")
    gqshp = [nc.dram_tensor(f"gqshp{p}", [PIECE_ROWS[p], D], f32, kind="Internal")
             for p in range(NPIECE)]
    gqcat = nc.dram_tensor("gqcat", [NTOT, D], f32, kind="Internal", addr_space="Shared")
    ysh = nc.dram_tensor("ysh", [P, NCC], f32, kind="Internal")
    yfull = nc.dram_tensor("yfull", [NCORE * P, NCC], f32, kind="Internal", addr_space="Shared")

    rg = [list(range(NCORE))]
    run_nch, run_calls = em["run_nch"], em["run_calls"]
    pend = np.cumsum(PIECE_BLK) - 1          # windows 12, 24, 36, 48

    with tile.TileContext(nc) as tc:
        with (
            tc.tile_pool(name="const", bufs=1) as cp,
            tc.tile_pool(name="big", bufs=1) as bp,
            tc.tile_pool(name="msgs", bufs=3) as mp,
            tc.tile_pool(name="oh", bufs=4) as ohp,
            tc.tile_pool(name="wrk", bufs=4) as wp,
            tc.tile_pool(name="zt", bufs=3) as zp,
            tc.tile_pool(name="ps_run", bufs=2, space="PSUM") as ps_run,
            tc.tile_pool(name="ps_t", bufs=2, space="PSUM") as ps_t,
            tc.tile_pool(name="ps_h", bufs=2, space="PSUM") as ps_h,
        ):
            nc.gpsimd.load_library(library_config.mlp)

            def load(pool, t, shape=None):
                tl = pool.tile(shape or list(t.shape), t.dtype, tag=t.name)
                nc.sync.dma_start(tl[:], t[:])
                return tl

            gidx_t = load(cp, gidx)
            dstloc_t = load(cp, dstloc)
            val_t = load(cp, val)
            iota_t = load(cp, iota)
            ident_t = load(cp, ident)
            onesr_t = load(cp, onesr)
            wself_t = [load(cp, w) for w in wself]
            wneigh_t = [load(cp, w) for w in wneigh]
            crow_t = [load(cp, w) for w in crow]
            abmat_t = load(cp, abmat)
            gqbias_t = load(cp, gqbias)
            mw0r_t = load(cp, mw0r)
            mw1b_t = load(cp, mw1b)
            mw2b_t = load(cp, mw2b)
            cu_t = load(cp, cu)
            cv_t = load(cp, cv)
            feat_t = load(cp, feat)
            mask_t = load(cp, maskr)

            curT = bp.tile([P, NSH], f32, tag="curT")
            nxtT = bp.tile([P, NSH], f32, tag="nxtT")
            nc.sync.dma_start(curT[:], xT[:])

            # zero gather bufs once: short calls leave stale tail chunks
            # whose one-hot is all-zero; 0*NaN would poison the psum
            for _ in range(3):
                g0 = mp.tile([P, GMAX // P, P], f32, tag="g")
                nc.vector.memset(g0[:], 0.0)
            ut = bp.tile([P, NCC, 64], f32, tag="ut")
            vt = bp.tile([P, NCC, 64], f32, tag="vt")

            for layer in range(2):
                tab = xcat if layer == 0 else hcat
                for w in range(NWIN):
                    wch = run_nch[w]
                    aggw = wp.tile([P, P], f32, tag="aggw")
                    if wch == 0:
                        nc.vector.memset(aggw[:], 0.0)
                    else:
                        ps = ps_run.tile([P, P], f32, tag="psw")
                        jw = 0
                        for (s0, n_idx) in run_calls[w]:
                            ncall = n_idx // P
                            g = mp.tile([P, GMAX // P, P], f32, tag="g")
                            nc.gpsimd.dma_gather(
                                g[:, :ncall, :], tab[HALF:NTOT, :],
                                gidx_t[:, s0 // 16 : (s0 + n_idx) // 16],
                                n_idx, n_idx, P, single_packet=False)
                            for cc in range(ncall):
                                ch = s0 // P + cc
                                oh = ohp.tile([P, P], f32, tag="oh")
                                nc.vector.tensor_scalar(
                                    oh[:], iota_t[:], dstloc_t[:, ch : ch + 1],
                                    val_t[:, ch : ch + 1], ALU.is_equal, ALU.mult)
                                nc.tensor.matmul(ps[:], lhsT=g[:, cc, :], rhs=oh[:],
                                                 start=(jw == 0), stop=(jw == wch - 1))
                                jw += 1
                        nc.scalar.activation(aggw[:], ps[:], AF.Copy)
                    nb = w * P
                    ph = ps_h.tile([P, P], f32, tag="ph")
                    nc.tensor.matmul(ph[:], lhsT=wneigh_t[layer][:], rhs=aggw[:],
                                     start=True, stop=False)
                    nc.tensor.matmul(ph[:], lhsT=wself_t[layer][:],
                                     rhs=curT[:, nb : nb + P], start=False, stop=False)
                    nc.tensor.matmul(ph[:], lhsT=crow_t[layer][:], rhs=onesr_t[:],
                                     start=False, stop=True)
                    tmp = wp.tile([P, P], f32, tag="tmp")
                    nc.vector.tensor_scalar(tmp[:], ph[:], SLOPE, None, ALU.mult)
                    nc.vector.tensor_tensor(nxtT[:, nb : nb + P], ph[:], tmp[:], ALU.max)
                    wp_ = next(i for i in range(NPIECE)
                               if PIECE_R0[i] <= nb < PIECE_R0[i] + PIECE_BLK[i] * P)
                    rb = nb - PIECE_R0[wp_]
                    if layer == 0:
                        pt = ps_t.tile([P, P], f32, tag="pt")
                        nc.tensor.transpose(pt[:], nxtT[:, nb : nb + P], ident_t[:])
                        hb = wp.tile([P, P], f32, tag="hb")
                        nc.scalar.activation(hb[:], pt[:], AF.Copy)
                        nc.sync.dma_start(hshp[wp_][rb : rb + P, :], hb[:])
                    else:
                        pg = ps_h.tile([P, P], f32, tag="ph")
                        nc.tensor.matmul(pg[:], lhsT=nxtT[:, nb : nb + P],
                                         rhs=abmat_t[:], start=True, stop=False)
                        nc.tensor.matmul(pg[:], lhsT=onesr_t[:], rhs=gqbias_t[:],
                                         start=False, stop=True)
                        gb = wp.tile([P, P], f32, tag="gb")
                        nc.scalar.activation(gb[:], pg[:], AF.Copy)
                        nc.sync.dma_start(gqshp[wp_][rb : rb + P, :], gb[:])
                    if w in pend:
                        pi = int(np.nonzero(pend == w)[0][0])
                        cb0 = PIECE_CATB[pi]
                        cb1 = cb0 + NCORE * PIECE_ROWS[pi]
                        if layer == 0:
                            nc.gpsimd.collective_compute(
                                "AllGather", ALU.bypass, replica_groups=rg,
                                ins=[hshp[pi][:].opt()], outs=[hcat[cb0:cb1, :].opt()])
                        else:
                            nc.gpsimd.collective_compute(
                                "AllGather", ALU.bypass, replica_groups=rg,
                                ins=[gqshp[pi][:].opt()], outs=[gqcat[cb0:cb1, :].opt()])
                if layer == 0:
                    curT, nxtT = nxtT, curT

            # ---- candidate gathers (u then v, interleaved by slot) ----
            for (s0, n_idx) in cm["calls"]:
                for (tl, idx_t, cofs) in ((ut, cu_t, 0), (vt, cv_t, 64)):
                    nc.gpsimd.dma_gather(
                        tl[:, s0 // P : (s0 + n_idx) // P, :],
                        gqcat[HALF:NTOT, cofs : cofs + 64],
                        idx_t[:, s0 // 16 : (s0 + n_idx) // 16],
                        n_idx, n_idx, 64, elem_step=P, single_packet=False)

            # ---- candidate MLP in 512-cand tiles ----
            ycol = bp.tile([P, NCC], f32, tag="ycol")
            for t0 in range(0, NCC, TGRP):
                tn = min(TGRP, NCC - t0)
                z1 = zp.tile([P, TGRP, 64], f32, tag="z1")
                for c in range(tn):
                    nc.vector.tensor_scalar(z1[:, c, :], mw0r_t[:],
                                            feat_t[:, t0 + c : t0 + c + 1],
                                            None, ALU.mult)
                nc.vector.tensor_tensor(z1[:, :tn, :], z1[:, :tn, :],
                                        ut[:, t0 : t0 + tn, :], ALU.add)
                nc.vector.tensor_tensor(z1[:, :tn, :], z1[:, :tn, :],
                                        vt[:, t0 : t0 + tn, :], ALU.add)
                zs = zp.tile([P, TGRP, 64], f32, tag="zs")
                nc.vector.tensor_scalar(zs[:, :tn, :], z1[:, :tn, :], SLOPE,
                                        None, ALU.mult)
                nc.vector.tensor_tensor(z1[:, :tn, :], z1[:, :tn, :],
                                        zs[:, :tn, :], ALU.max)
                z1t = zp.tile([65, TGRP * P], f32, tag="z1t")
                nc.vector.memset(z1t[64:65, :], 1.0)
                for c in range(tn):
                    pt2 = ps_t.tile([64, P], f32, tag="pt")
                    nc.tensor.transpose(pt2[:], z1[:, c, :], ident_t[:])
                    nc.scalar.activation(z1t[0:64, c * P : (c + 1) * P], pt2[:], AF.Copy)
                ps2 = ps_run.tile([64, TGRP * P], f32, tag="ps2")
                nc.tensor.matmul(ps2[:, : tn * P], lhsT=mw1b_t[:],
                                 rhs=z1t[:, : tn * P], start=True, stop=True)
                z2t = zp.tile([65, TGRP * P], f32, tag="z2t")
                nc.vector.memset(z2t[64:65, :], 1.0)
                nc.vector.tensor_scalar(z2t[0:64, : tn * P], ps2[:, : tn * P],
                                        SLOPE, None, ALU.mult)
                nc.vector.tensor_tensor(z2t[0:64, : tn * P], z2t[0:64, : tn * P],
                                        ps2[:, : tn * P], ALU.max)
                for c in range(tn):
                    py = ps_h.tile([P, 1], f32, tag="ph")
                    nc.tensor.matmul(py[:], lhsT=z2t[:, c * P : (c + 1) * P],
                                     rhs=mw2b_t[:], start=True, stop=True)
                    nc.scalar.activation(ycol[:, t0 + c : t0 + c + 1], py[:], AF.Copy)

            nc.sync.dma_start(y_out[:], ycol[:])
            ym = wp.tile([P, NCC], f32, tag="ym")
            nc.vector.tensor_tensor(ym[:], ycol[:], mask_t[:], ALU.add)
            nc.sync.dma_start(ysh[:], ym[:])
            nc.gpsimd.collective_compute(
                "AllGather", ALU.bypass, replica_groups=rg,
                ins=[ysh[:].opt()], outs=[yfull[:].opt()])
            # ---- softmax ----
            ncols = NCORE * CSLOT // P
            yf = bp.tile([P, ncols], f32, tag="yf")
            nc.sync.dma_start(yf[:], yfull[:].rearrange("a b -> (a b)")
                              .rearrange("(p c) -> p c", p=P))
            rmax = wp.tile([P, 1], f32, tag="rmax")
            nc.vector.tensor_reduce(rmax[:], yf[:], mybir.AxisListType.X, ALU.max)
            gmax = wp.tile([P, 1], f32, tag="gmax")
            nc.gpsimd.partition_all_reduce(gmax[:], rmax[:], P,
                                           bass_isa.ReduceOp.max)
            ngmax = wp.tile([P, 1], f32, tag="ngmax")
            nc.vector.tensor_scalar(ngmax[:], gmax[:], -1.0, None, ALU.mult)
            ef = bp.tile([P, ncols], f32, tag="ef")
            se = wp.tile([P, 1], f32, tag="se")
            nc.scalar.activation(ef[:], yf[:], AF.Exp, bias=ngmax[:, 0:1],
                                 accum_out=se[:])
            stot = wp.tile([P, 1], f32, tag="stot")
            nc.gpsimd.partition_all_reduce(stot[:], se[:], P, bass_isa.ReduceOp.add)
            invs = wp.tile([P, 1], f32, tag="invs")
            nc.vector.reciprocal(invs[:], stot[:])
            pf = bp.tile([P, ncols], f32, tag="pf")
            nc.vector.tensor_scalar(pf[:], ef[:], invs[:, 0:1], None, ALU.mult)
            nc.sync.dma_start(p_out[:], pf[:])
    nc.compile()
    return nc


def kernel(x, src, dst, cand_u, cand_v, cand_feat,
           w_self0, w_neigh0, b0, gamma0, beta0, rm0, rv0,
           w_self1, w_neigh1, b1, gamma1, beta1, rm1, rv1,
           mw0, mb0, mw1, mb1, mw2, mb2):
    x = np.asarray(x, np.float32)
    src = np.asarray(src, np.int64)
    dst = np.asarray(dst, np.int64)
    cand_u = np.asarray(cand_u, np.int64)
    cand_v = np.asarray(cand_v, np.int64)
    cand_feat = np.asarray(cand_feat, np.float32)

    deg = np.bincount(dst, minlength=N).astype(np.float32)
    invdeg = 1.0 / np.maximum(deg, 1.0)
    em, edata = _prep_edges(src, dst, invdeg)
    cm, cdata = _prep_cands(cand_u, cand_v, cand_feat)

    xpad = np.zeros((NTOT, D), np.float32)
    xpad[:N] = x
    iota = np.tile(np.arange(P, dtype=np.float32), (P, 1))
    ident = np.eye(P, dtype=np.float32)
    onesr = np.ones((1, P), np.float32)

    com = {"iota": iota, "ident": ident, "onesr": onesr}
    # piece-major concatenated x table
    xsh = xpad.reshape(NCORE, NSH, D)
    xcat = np.zeros((NTOT, D), np.float32)
    for p in range(NPIECE):
        r0, rows, cb = PIECE_R0[p], PIECE_ROWS[p], PIECE_CATB[p]
        xcat[cb : cb + NCORE * rows] = xsh[:, r0 : r0 + rows, :].reshape(-1, D)
    com["xcat"] = xcat
    for l, (ws, wn, b, ga, be, rme, rve) in enumerate(
        ((w_self0, w_neigh0, b0, gamma0, beta0, rm0, rv0),
         (w_self1, w_neigh1, b1, gamma1, beta1, rm1, rv1))):
        a = (ga / np.sqrt(rve + BN_EPS)).astype(np.float32)
        com[f"wself{l}"] = (ws * a[None, :]).astype(np.float32)
        com[f"wneigh{l}"] = (wn * a[None, :]).astype(np.float32)
        com[f"crow{l}"] = (a * (b - rme) + be).astype(np.float32)[None, :]
    com["abmat"] = np.concatenate(
        [np.asarray(mw0[0:128], np.float32), np.asarray(mw0[128:256], np.float32)], axis=1)
    com["gqbias"] = np.concatenate(
        [np.zeros(64, np.float32), np.asarray(mb0, np.float32)])[None, :]
    com["mw0r"] = np.tile(np.asarray(mw0[256], np.float32), (P, 1))
    com["mw1b"] = np.concatenate(
        [np.asarray(mw1, np.float32), np.asarray(mb1, np.float32)[None, :]], axis=0)
    com["mw2b"] = np.concatenate(
        [np.asarray(mw2, np.float32),
         np.asarray(mb2, np.float32).reshape(1, 1)], axis=0)

    nc = _build_nc(em, cm)
    in_maps = []
    for k in range(NCORE):
        m = dict(com)
        m["xT"] = xpad[k * NSH : (k + 1) * NSH].T.copy()
        m["gidx"] = edata[k]["gidx"]
        m["dstloc"] = edata[k]["dstloc"]
        m["val"] = edata[k]["val"]
        m["cu"] = cdata[k]["cu"]
        m["cv"] = cdata[k]["cv"]
        m["feat"] = cdata[k]["feat"]
        m["maskr"] = cdata[k]["mask"]
        in_maps.append(m)
    import os
    trace = bool(os.environ.get("KERNEL_TRACE"))
    if trace:
        import types
        import ctypes
        if "antenv.axon_hooks" not in sys.modules:
            try:
                import antenv
                from trn_agent_boot.trn_boot import _ntff_profile_via_ctypes
                mod = types.ModuleType("antenv.axon_hooks")
                hook = [_ntff_profile_via_ctypes("/opt/axon/libaxon_pjrt.so")]
                mod.set_axon_ntff_profile_hook = lambda h: hook.__setitem__(0, h)
                mod.get_axon_ntff_profile_hook = lambda: hook[0]
                sys.modules["antenv.axon_hooks"] = mod
                antenv.axon_hooks = mod
            except Exception:
                trace = False
    res = run_bass_kernel_spmd(nc, in_maps, core_ids=list(range(NCORE)),
                               trace=trace,
                               tmpdir=os.environ.get("KERNEL_TRACE_DIR"))
    if trace and res.exec_time_ns is not None:
        print(f"HW exec time: {res.exec_time_ns} ns")
    y_all = np.zeros(C, np.float32)
    p_all = np.zeros(C, np.float32)
    ncc = cm["ncc"]
    p_lin = res.results[0]["p_out"].ravel()   # global order: k, p, c
    for k in range(NCORE):
        sm = cdata[k]["slotmap"]
        valid = sm >= 0
        j = np.nonzero(valid)[0]              # slot j = c*128 + p
        yk = res.results[k]["y_out"]          # [128, NCC] -> value at [j%128, j//128]
        y_all[sm[valid]] = yk[j % P, j // P]
        gs = k * cm["cslot"] + (j % P) * ncc + (j // P)
        p_all[sm[valid]] = p_lin[gs]
    return y_all[:, None], p_all[:, None]


# revision 15
# speedup vs baseline: 1.4156x; 1.0464x over previous
"""Trainium2 Bass kernel for nn_PolicyNetwork3 (2-layer GraphSAGE + edge-MLP).

Design (8 NeuronCores, SPMD single NEFF):
- dst-sharded aggregation: core k owns node block [6272k, 6272k+6272).
- Node tables stored PIECE-MAJOR in one concatenated [50176, 128] table;
  the inter-layer AllGather streams piece-wise into row slices of it,
  overlapped with the window loop. Gather indices are SIGNED int16 offsets
  from the table midpoint (the Q7 address math sign-extends), so one call
  covers the whole table.
- Edges bucketed per 128-dst window; ONE dma_gather call per window
  (amortizes the ~3us per-call Q7 overhead). Every call's last index is
  kept non-negative so the ucode's trailing-negative trim never fires.
- segment-sum per window via one-hot matmuls accumulating in PSUM
  (one-hot weighted by 1/max(deg,1), built on DVE).
- Linear+BN (folded) fused per window, flipped to produce [feat, node];
  layer 1 fuses the candidate-MLP first-layer projection (gq tables).
- Candidate MLP runs in 512-candidate tiles pipelined with the u/v
  gathers; global softmax on-device after an AllGather of y.
"""

import sys

sys.path.insert(0, "/opt/trn_rl_repo")
sys.path.insert(0, "/root/.axon_site")

import numpy as np

import concourse.bacc as bacc
import concourse.bass as bass
import concourse.bass_isa as bass_isa
import concourse.mybir as mybir
import concourse.tile as tile
from concourse import library_config
from concourse.bass_utils import run_bass_kernel_spmd

P = 128
N, E, C = 50000, 800000, 100000
D = 128
NCORE = 8
NSH = 6272          # nodes per core shard (49 blocks of 128)
NTOT = NSH * NCORE  # 50176 padded node table
NWIN = NSH // P     # 49 windows (= 128-node blocks)
HALF = NTOT // 2    # gather base row (signed idx16 offsets from here)
CSH = C // NCORE    # 12500 candidates per core
GMAX = 2560         # max idxs per dma_gather call
TGRP = 4            # candidate chunks per MLP tile (512 cands)
NPIECE = 4
PIECE_BLK = [13, 12, 12, 12]          # blocks per piece
PIECE_R0 = [0, 1664, 3200, 4736]      # first local row of each piece
PIECE_ROWS = [1664, 1536, 1536, 1536]
PIECE_CATB = [0, 13312, 25600, 37888]  # piece base row in the cat table
BN_EPS = 1e-5
SLOPE = 0.01
F32 = mybir.dt.float32
BF16 = mybir.dt.bfloat16
I16 = mybir.dt.int16
AF = mybir.ActivationFunctionType
ALU = mybir.AluOpType


def _wrap16(idx_lin):
    """[n] -> [128, n/16] int16 in the dma_gather wrapped+replicated layout."""
    n = idx_lin.shape[0]
    assert n % 16 == 0
    w = idx_lin.reshape(n // 16, 16).T.astype(np.int16)
    return np.tile(w, (8, 1)).copy()


def gidx_to_cols(arr):
    """[nslot] -> [128, nchunk] with slot i at [i%128, i//128]."""
    n = arr.shape[0]
    return arr.reshape(n // P, P).T.copy()


def _cat_row(src):
    """global node id -> row in the piece-major cat table."""
    src = np.asarray(src)
    k = src // NSH
    r = src % NSH
    p = np.searchsorted(np.asarray(PIECE_R0[1:]), r, side="right")
    return (np.asarray(PIECE_CATB)[p] + k * np.asarray(PIECE_ROWS)[p]
            + (r - np.asarray(PIECE_R0)[p]))


def _split_calls(p0, p1):
    """slot range -> near-equal calls of <= GMAX idxs (multiples of 128)."""
    total = p1 - p0
    ncall = -(-total // GMAX)
    per = -(-(total // P) // ncall) * P
    calls = []
    q = p0
    while q < p1:
        n = min(per, p1 - q)
        calls.append((q, n))
        q += n
    return calls


def _guard_calls(idxmats, calls):
    """Ensure the last slot of every call has all idx matrices >= 0 there,
    swapping in a suitable slot from the same call (or whole range)."""
    nmat = len(idxmats)
    ok = np.ones(idxmats[0].shape[0], bool)
    for m in idxmats:
        ok &= (m >= 0)
    lasts = {q + n - 1 for (q, n) in calls}
    for (q, n) in calls:
        last = q + n - 1
        if ok[last]:
            continue
        cand = [j for j in range(q, q + n - 1) if ok[j] and j not in lasts]
        assert cand, "no non-negative slot available for call guard"
        j = cand[-1]
        for m in idxmats:
            m[last], m[j] = m[j], m[last]
    return idxmats


def _prep_edges(src, dst, invdeg):
    """Per-window buckets of 128-edge chunks, uniform across cores."""
    core = np.minimum(dst // NSH, NCORE - 1)
    winl = (dst - core * NSH) // P
    g = _cat_row(src)
    key = core * NWIN + winl
    nbkt = NCORE * NWIN
    order = np.argsort(key, kind="stable")
    cnt = np.bincount(key, minlength=nbkt).reshape(NCORE, NWIN)
    nch_u = (-(-cnt // P)).max(axis=0)       # [NWIN] uniform chunk counts
    run_off = np.zeros(NWIN + 1, np.int64)
    np.cumsum(nch_u, out=run_off[1:])
    tot_ch = int(run_off[-1])
    nslot = tot_ch * P

    gidx = np.zeros((NCORE, nslot), np.int16)        # pads read row HALF
    dstloc = np.full((NCORE, nslot), -5.0, np.float32)
    val = np.zeros((NCORE, nslot), np.float32)
    bstart = np.zeros(nbkt + 1, np.int64)
    np.cumsum(np.bincount(key, minlength=nbkt), out=bstart[1:])
    run_calls = [_split_calls(int(run_off[w]) * P, int(run_off[w + 1]) * P)
                 for w in range(NWIN)]
    for k in range(NCORE):
        for w in range(NWIN):
            b = k * NWIN + w
            e0, e1 = bstart[b], bstart[b + 1]
            n = e1 - e0
            if n == 0:
                continue
            sl = order[e0:e1]
            sl = sl[np.argsort(g[sl], kind="stable")]
            pos = int(run_off[w]) * P
            gidx[k, pos : pos + n] = (g[sl] - HALF).astype(np.int16)
            dstloc[k, pos : pos + n] = (dst[sl] % P).astype(np.float32)
            val[k, pos : pos + n] = invdeg[dst[sl]]
        # keep last slot of every call non-negative (trim guard); swap
        # dstloc/val along with the index
        for w in range(NWIN):
            calls = run_calls[w]
            lasts = [q + n - 1 for (q, n) in calls]
            for (q, n) in calls:
                last = q + n - 1
                if gidx[k, last] >= 0:
                    continue
                p0, p1 = int(run_off[w]) * P, int(run_off[w + 1]) * P
                cand = [j for j in range(p1 - 1, p0 - 1, -1)
                        if gidx[k, j] >= 0 and j not in lasts]
                assert cand, "no non-negative index in window bucket"
                j = cand[0]
                for m in (gidx, dstloc, val):
                    m[k, last], m[k, j] = m[k, j], m[k, last]
    meta = dict(run_off=run_off, run_nch=[int(x) for x in nch_u],
                tot_ch=tot_ch, nslot=nslot, run_calls=run_calls)
    data = [dict(gidx=_wrap16(gidx[k]),
                 dstloc=gidx_to_cols(dstloc[k]),
                 val=gidx_to_cols(val[k])) for k in range(NCORE)]
    return meta, data


def _prep_cands(cand_u, cand_v, cand_feat):
    """Shard candidates per core, sort by u row, pad to uniform chunks."""
    gu = _cat_row(cand_u) - HALF
    gv = _cat_row(cand_v) - HALF
    ncc = -(-CSH // P)
    cslot = ncc * P
    cu = np.zeros((NCORE, cslot), np.int16)
    cv = np.zeros((NCORE, cslot), np.int16)
    ft = np.zeros((NCORE, cslot), np.float32)
    mask = np.full((NCORE, cslot), -1e30, np.float32)
    slotmap = np.full((NCORE, cslot), -1, np.int64)
    calls = _split_calls(0, cslot)
    for k in range(NCORE):
        ids = np.arange(k * CSH, (k + 1) * CSH)
        ids = ids[np.argsort(gu[ids], kind="stable")]
        cu[k, :CSH] = gu[ids].astype(np.int16)
        cv[k, :CSH] = gv[ids].astype(np.int16)
        ft[k, :CSH] = cand_feat[ids, 0]
        mask[k, :CSH] = 0.0
        slotmap[k, :CSH] = ids
        # joint guard: last slot of each call needs cu>=0 AND cv>=0
        both = (cu[k] >= 0) & (cv[k] >= 0)
        lasts = [q + n - 1 for (q, n) in calls]
        for (q, n) in calls:
            last = q + n - 1
            if both[last]:
                continue
            cand = [j for j in range(q + n - 2, q - 1, -1)
                    if both[j] and j not in lasts]
            if not cand:
                cand = [j for j in range(cslot - 1, -1, -1)
                        if both[j] and j not in lasts]
            assert cand, "no jointly non-negative candidate for call guard"
            j = cand[0]
            for m in (cu, cv, ft, mask):
                m[k, last], m[k, j] = m[k, j], m[k, last]
            slotmap[k, last], slotmap[k, j] = slotmap[k, j], slotmap[k, last]
            both[last], both[j] = both[j], both[last]
    meta = dict(ncc=ncc, cslot=cslot, calls=calls)
    data = [dict(cu=_wrap16(cu[k]), cv=_wrap16(cv[k]),
                 feat=gidx_to_cols(ft[k]), mask=gidx_to_cols(mask[k]),
                 slotmap=slotmap[k]) for k in range(NCORE)]
    return meta, data


def _build_nc(em, cm):
    nc = bacc.Bacc("TRN2", target_bir_lowering=False, debug=False,
                   num_devices=NCORE)
    f32 = F32
    TOTCH, NSLOT = em["tot_ch"], em["nslot"]
    NCC, CSLOT = cm["ncc"], cm["cslot"]

    # ---- external inputs ----
    xcat = nc.dram_tensor("xcat", [NTOT, D], BF16, kind="ExternalInput")
    xT = nc.dram_tensor("xT", [P, NSH], f32, kind="ExternalInput")
    gidx = nc.dram_tensor("gidx", [P, NSLOT // 16], I16, kind="ExternalInput")
    dstloc = nc.dram_tensor("dstloc", [P, TOTCH], f32, kind="ExternalInput")
    val = nc.dram_tensor("val", [P, TOTCH], f32, kind="ExternalInput")
    wself = [nc.dram_tensor(f"wself{l}", [D, D], f32, kind="ExternalInput") for l in range(2)]
    wneigh = [nc.dram_tensor(f"wneigh{l}", [D, D], f32, kind="ExternalInput") for l in range(2)]
    crow = [nc.dram_tensor(f"crow{l}", [1, D], f32, kind="ExternalInput") for l in range(2)]
    iota = nc.dram_tensor("iota", [P, P], f32, kind="ExternalInput")
    ident = nc.dram_tensor("ident", [P, P], f32, kind="ExternalInput")
    onesr = nc.dram_tensor("onesr", [1, P], f32, kind="ExternalInput")
    abmat = nc.dram_tensor("abmat", [D, D], f32, kind="ExternalInput")
    gqbias = nc.dram_tensor("gqbias", [1, D], f32, kind="ExternalInput")
    mw0r = nc.dram_tensor("mw0r", [P, 64], f32, kind="ExternalInput")
    mw1b = nc.dram_tensor("mw1b", [65, 64], f32, kind="ExternalInput")
    mw2b = nc.dram_tensor("mw2b", [65, 1], f32, kind="ExternalInput")
    cu = nc.dram_tensor("cu", [P, CSLOT // 16], I16, kind="ExternalInput")
    cv = nc.dram_tensor("cv", [P, CSLOT // 16], I16, kind="ExternalInput")
    feat = nc.dram_tensor("feat", [P, NCC], f32, kind="ExternalInput")
    maskr = nc.dram_tensor("maskr", [P, NCC], f32, kind="ExternalInput")
    # ---- outputs ----
    y_out = nc.dram_tensor("y_out", [P, NCC], f32, kind="ExternalOutput")
    p_out = nc.dram_tensor("p_out", [P, NCORE * CSLOT // P], f32, kind="ExternalOutput")
    # ---- internal DRAM ----
    hshp = [nc.dram_tensor(f"hshp{p}", [PIECE_ROWS[p], D], BF16, kind="Internal")
            for p in range(NPIECE)]
    hcat = nc.dram_tensor("hcat", [NTOT, D], BF16, kind="Internal", addr_space="Shared")
    gqshp = [nc.dram_tensor(f"gqshp{p}", [PIECE_ROWS[p], D], BF16, kind="Internal")
             for p in range(NPIECE)]
    gqcat = nc.dram_tensor("gqcat", [NTOT, D], BF16, kind="Internal", addr_space="Shared")
    ysh = nc.dram_tensor("ysh", [P, NCC], f32, kind="Internal")
    yfull = nc.dram_tensor("yfull", [NCORE * P, NCC], f32, kind="Internal", addr_space="Shared")

    rg = [list(range(NCORE))]
    run_nch, run_calls = em["run_nch"], em["run_calls"]
    pend = np.cumsum(PIECE_BLK) - 1          # windows 12, 24, 36, 48

    with tile.TileContext(nc) as tc:
        with (
            tc.tile_pool(name="const", bufs=1) as cp,
            tc.tile_pool(name="big", bufs=1) as bp,
            tc.tile_pool(name="msgs", bufs=3) as mp,
            tc.tile_pool(name="oh", bufs=4) as ohp,
            tc.tile_pool(name="wrk", bufs=4) as wp,
            tc.tile_pool(name="zt", bufs=3) as zp,
            tc.tile_pool(name="ps_run", bufs=2, space="PSUM") as ps_run,
            tc.tile_pool(name="ps_t", bufs=2, space="PSUM") as ps_t,
            tc.tile_pool(name="ps_h", bufs=2, space="PSUM") as ps_h,
        ):
            nc.gpsimd.load_library(library_config.mlp)

            def load(pool, t, shape=None):
                tl = pool.tile(shape or list(t.shape), t.dtype, tag=t.name)
                nc.sync.dma_start(tl[:], t[:])
                return tl

            gidx_t = load(cp, gidx)
            dstloc_t = load(cp, dstloc)
            val_t = load(cp, val)
            iota_t = load(cp, iota)
            ident_t = load(cp, ident)
            onesr_t = load(cp, onesr)
            wself_t = [load(cp, w) for w in wself]
            wneigh_t = [load(cp, w) for w in wneigh]
            crow_t = [load(cp, w) for w in crow]
            abmat_t = load(cp, abmat)
            gqbias_t = load(cp, gqbias)
            mw0r_t = load(cp, mw0r)
            mw1b_t = load(cp, mw1b)
            mw2b_t = load(cp, mw2b)
            cu_t = load(cp, cu)
            cv_t = load(cp, cv)
            feat_t = load(cp, feat)
            mask_t = load(cp, maskr)

            curT = bp.tile([P, NSH], f32, tag="curT")
            nxtT = bp.tile([P, NSH], f32, tag="nxtT")
            nc.sync.dma_start(curT[:], xT[:])

            # zero gather bufs once: short calls leave stale tail chunks
            # whose one-hot is all-zero; 0*NaN would poison the psum
            for _ in range(3):
                g0 = mp.tile([P, GMAX // P, P], BF16, tag="g")
                nc.vector.memset(g0[:], 0.0)
            ut = bp.tile([P, NCC, D], BF16, tag="ut")
            vt = bp.tile([P, NCC, D], BF16, tag="vt")

            for layer in range(2):
                tab = xcat if layer == 0 else hcat
                for w in range(NWIN):
                    wch = run_nch[w]
                    aggw = wp.tile([P, P], f32, tag="aggw")
                    if wch == 0:
                        nc.vector.memset(aggw[:], 0.0)
                    else:
                        ps = ps_run.tile([P, P], f32, tag="psw")
                        jw = 0
                        for (s0, n_idx) in run_calls[w]:
                            ncall = n_idx // P
                            g = mp.tile([P, GMAX // P, P], BF16, tag="g")
                            nc.gpsimd.dma_gather(
                                g[:, :ncall, :], tab[HALF:NTOT, :],
                                gidx_t[:, s0 // 16 : (s0 + n_idx) // 16],
                                n_idx, n_idx, P, single_packet=False)
                            for cc in range(ncall):
                                ch = s0 // P + cc
                                oh = ohp.tile([P, P], BF16, tag="oh")
                                nc.vector.tensor_scalar(
                                    oh[:], iota_t[:], dstloc_t[:, ch : ch + 1],
                                    val_t[:, ch : ch + 1], ALU.is_equal, ALU.mult)
                                nc.tensor.matmul(ps[:], lhsT=g[:, cc, :], rhs=oh[:],
                                                 start=(jw == 0), stop=(jw == wch - 1))
                                jw += 1
                        nc.scalar.activation(aggw[:], ps[:], AF.Copy)
                    nb = w * P
                    ph = ps_h.tile([P, P], f32, tag="ph")
                    nc.tensor.matmul(ph[:], lhsT=wneigh_t[layer][:], rhs=aggw[:],
                                     start=True, stop=False)
                    nc.tensor.matmul(ph[:], lhsT=wself_t[layer][:],
                                     rhs=curT[:, nb : nb + P], start=False, stop=False)
                    nc.tensor.matmul(ph[:], lhsT=crow_t[layer][:], rhs=onesr_t[:],
                                     start=False, stop=True)
                    tmp = wp.tile([P, P], f32, tag="tmp")
                    nc.vector.tensor_scalar(tmp[:], ph[:], SLOPE, None, ALU.mult)
                    nc.vector.tensor_tensor(nxtT[:, nb : nb + P], ph[:], tmp[:], ALU.max)
                    wp_ = next(i for i in range(NPIECE)
                               if PIECE_R0[i] <= nb < PIECE_R0[i] + PIECE_BLK[i] * P)
                    rb = nb - PIECE_R0[wp_]
                    if layer == 0:
                        pt = ps_t.tile([P, P], f32, tag="pt")
                        nc.tensor.transpose(pt[:], nxtT[:, nb : nb + P], ident_t[:])
                        hb = wp.tile([P, P], BF16, tag="hb")
                        nc.scalar.activation(hb[:], pt[:], AF.Copy)
                        nc.sync.dma_start(hshp[wp_][rb : rb + P, :], hb[:])
                    else:
                        pg = ps_h.tile([P, P], f32, tag="ph")
                        nc.tensor.matmul(pg[:], lhsT=nxtT[:, nb : nb + P],
                                         rhs=abmat_t[:], start=True, stop=False)
                        nc.tensor.matmul(pg[:], lhsT=onesr_t[:], rhs=gqbias_t[:],
                                         start=False, stop=True)
                        gb = wp.tile([P, P], BF16, tag="gb")
                        nc.scalar.activation(gb[:], pg[:], AF.Copy)
                        nc.sync.dma_start(gqshp[wp_][rb : rb + P, :], gb[:])
                    if w in pend:
                        pi = int(np.nonzero(pend == w)[0][0])
                        cb0 = PIECE_CATB[pi]
                        cb1 = cb0 + NCORE * PIECE_ROWS[pi]
                        if layer == 0:
                            nc.gpsimd.collective_compute(
                                "AllGather", ALU.bypass, replica_groups=rg,
                                ins=[hshp[pi][:].opt()], outs=[hcat[cb0:cb1, :].opt()])
                        else:
                            nc.gpsimd.collective_compute(
                                "AllGather", ALU.bypass, replica_groups=rg,
                                ins=[gqshp[pi][:].opt()], outs=[gqcat[cb0:cb1, :].opt()])
                if layer == 0:
                    curT, nxtT = nxtT, curT

            # ---- candidate gathers (u then v, interleaved by slot) ----
            for (s0, n_idx) in cm["calls"]:
                for (tl, idx_t) in ((ut, cu_t), (vt, cv_t)):
                    nc.gpsimd.dma_gather(
                        tl[:, s0 // P : (s0 + n_idx) // P, :],
                        gqcat[HALF:NTOT, :],
                        idx_t[:, s0 // 16 : (s0 + n_idx) // 16],
                        n_idx, n_idx, D, single_packet=False)

            # ---- candidate MLP in 512-cand tiles ----
            ycol = bp.tile([P, NCC], f32, tag="ycol")
            for t0 in range(0, NCC, TGRP):
                tn = min(TGRP, NCC - t0)
                z1 = zp.tile([P, TGRP, 64], f32, tag="z1")
                for c in range(tn):
                    nc.vector.tensor_scalar(z1[:, c, :], mw0r_t[:],
                                            feat_t[:, t0 + c : t0 + c + 1],
                                            None, ALU.mult)
                nc.vector.tensor_tensor(z1[:, :tn, :], z1[:, :tn, :],
                                        ut[:, t0 : t0 + tn, 0:64], ALU.add)
                nc.vector.tensor_tensor(z1[:, :tn, :], z1[:, :tn, :],
                                        vt[:, t0 : t0 + tn, 64:128], ALU.add)
                zs = zp.tile([P, TGRP, 64], f32, tag="zs")
                nc.vector.tensor_scalar(zs[:, :tn, :], z1[:, :tn, :], SLOPE,
                                        None, ALU.mult)
                nc.vector.tensor_tensor(z1[:, :tn, :], z1[:, :tn, :],
                                        zs[:, :tn, :], ALU.max)
                z1t = zp.tile([65, TGRP * P], f32, tag="z1t")
                nc.vector.memset(z1t[64:65, :], 1.0)
                for c in range(tn):
                    pt2 = ps_t.tile([64, P], f32, tag="pt")
                    nc.tensor.transpose(pt2[:], z1[:, c, :], ident_t[:])
                    nc.scalar.activation(z1t[0:64, c * P : (c + 1) * P], pt2[:], AF.Copy)
                ps2 = ps_run.tile([64, TGRP * P], f32, tag="ps2")
                nc.tensor.matmul(ps2[:, : tn * P], lhsT=mw1b_t[:],
                                 rhs=z1t[:, : tn * P], start=True, stop=True)
                z2t = zp.tile([65, TGRP * P], f32, tag="z2t")
                nc.vector.memset(z2t[64:65, :], 1.0)
                nc.vector.tensor_scalar(z2t[0:64, : tn * P], ps2[:, : tn * P],
                                        SLOPE, None, ALU.mult)
                nc.vector.tensor_tensor(z2t[0:64, : tn * P], z2t[0:64, : tn * P],
                                        ps2[:, : tn * P], ALU.max)
                for c in range(tn):
                    py = ps_h.tile([P, 1], f32, tag="ph")
                    nc.tensor.matmul(py[:], lhsT=z2t[:, c * P : (c + 1) * P],
                                     rhs=mw2b_t[:], start=True, stop=True)
                    nc.scalar.activation(ycol[:, t0 + c : t0 + c + 1], py[:], AF.Copy)

            nc.sync.dma_start(y_out[:], ycol[:])
            ym = wp.tile([P, NCC], f32, tag="ym")
            nc.vector.tensor_tensor(ym[:], ycol[:], mask_t[:], ALU.add)
            nc.sync.dma_start(ysh[:], ym[:])
            nc.gpsimd.collective_compute(
                "AllGather", ALU.bypass, replica_groups=rg,
                ins=[ysh[:].opt()], outs=[yfull[:].opt()])
            # ---- softmax ----
            ncols = NCORE * CSLOT // P
            yf = bp.tile([P, ncols], f32, tag="yf")
            nc.sync.dma_start(yf[:], yfull[:].rearrange("a b -> (a b)")
                              .rearrange("(p c) -> p c", p=P))
            rmax = wp.tile([P, 1], f32, tag="rmax")
            nc.vector.tensor_reduce(rmax[:], yf[:], mybir.AxisListType.X, ALU.max)
            gmax = wp.tile([P, 1], f32, tag="gmax")
            nc.gpsimd.partition_all_reduce(gmax[:], rmax[:], P,
                                           bass_isa.ReduceOp.max)
            ngmax = wp.tile([P, 1], f32, tag="ngmax")
            nc.vector.tensor_scalar(ngmax[:], gmax[:], -1.0, None, ALU.mult)
            ef = bp.tile([P, ncols], f32, tag="ef")
            se = wp.tile([P, 1], f32, tag="se")
            nc.scalar.activation(ef[:], yf[:], AF.Exp, bias=ngmax[:, 0:1],
                                 accum_out=se[:])
            stot = wp.tile([P, 1], f32, tag="stot")
            nc.gpsimd.partition_all_reduce(stot[:], se[:], P, bass_isa.ReduceOp.add)
            invs = wp.tile([P, 1], f32, tag="invs")
            nc.vector.reciprocal(invs[:], stot[:])
            pf = bp.tile([P, ncols], f32, tag="pf")
            nc.vector.tensor_scalar(pf[:], ef[:], invs[:, 0:1], None, ALU.mult)
            nc.sync.dma_start(p_out[:], pf[:])
    nc.compile()
    return nc


def kernel(x, src, dst, cand_u, cand_v, cand_feat,
           w_self0, w_neigh0, b0, gamma0, beta0, rm0, rv0,
           w_self1, w_neigh1, b1, gamma1, beta1, rm1, rv1,
           mw0, mb0, mw1, mb1, mw2, mb2):
    x = np.asarray(x, np.float32)
    src = np.asarray(src, np.int64)
    dst = np.asarray(dst, np.int64)
    cand_u = np.asarray(cand_u, np.int64)
    cand_v = np.asarray(cand_v, np.int64)
    cand_feat = np.asarray(cand_feat, np.float32)

    deg = np.bincount(dst, minlength=N).astype(np.float32)
    invdeg = 1.0 / np.maximum(deg, 1.0)
    em, edata = _prep_edges(src, dst, invdeg)
    cm, cdata = _prep_cands(cand_u, cand_v, cand_feat)

    xpad = np.zeros((NTOT, D), np.float32)
    xpad[:N] = x
    iota = np.tile(np.arange(P, dtype=np.float32), (P, 1))
    ident = np.eye(P, dtype=np.float32)
    onesr = np.ones((1, P), np.float32)

    com = {"iota": iota, "ident": ident, "onesr": onesr}
    # piece-major concatenated x table
    import ml_dtypes
    xsh = xpad.reshape(NCORE, NSH, D)
    xcat = np.zeros((NTOT, D), ml_dtypes.bfloat16)
    for p in range(NPIECE):
        r0, rows, cb = PIECE_R0[p], PIECE_ROWS[p], PIECE_CATB[p]
        xcat[cb : cb + NCORE * rows] = xsh[:, r0 : r0 + rows, :].reshape(-1, D)
    com["xcat"] = xcat
    for l, (ws, wn, b, ga, be, rme, rve) in enumerate(
        ((w_self0, w_neigh0, b0, gamma0, beta0, rm0, rv0),
         (w_self1, w_neigh1, b1, gamma1, beta1, rm1, rv1))):
        a = (ga / np.sqrt(rve + BN_EPS)).astype(np.float32)
        com[f"wself{l}"] = (ws * a[None, :]).astype(np.float32)
        com[f"wneigh{l}"] = (wn * a[None, :]).astype(np.float32)
        com[f"crow{l}"] = (a * (b - rme) + be).astype(np.float32)[None, :]
    com["abmat"] = np.concatenate(
        [np.asarray(mw0[0:128], np.float32), np.asarray(mw0[128:256], np.float32)], axis=1)
    com["gqbias"] = np.concatenate(
        [np.zeros(64, np.float32), np.asarray(mb0, np.float32)])[None, :]
    com["mw0r"] = np.tile(np.asarray(mw0[256], np.float32), (P, 1))
    com["mw1b"] = np.concatenate(
        [np.asarray(mw1, np.float32), np.asarray(mb1, np.float32)[None, :]], axis=0)
    com["mw2b"] = np.concatenate(
        [np.asarray(mw2, np.float32),
         np.asarray(mb2, np.float32).reshape(1, 1)], axis=0)

    nc = _build_nc(em, cm)
    in_maps = []
    for k in range(NCORE):
        m = dict(com)
        m["xT"] = xpad[k * NSH : (k + 1) * NSH].T.copy()
        m["gidx"] = edata[k]["gidx"]
        m["dstloc"] = edata[k]["dstloc"]
        m["val"] = edata[k]["val"]
        m["cu"] = cdata[k]["cu"]
        m["cv"] = cdata[k]["cv"]
        m["feat"] = cdata[k]["feat"]
        m["maskr"] = cdata[k]["mask"]
        in_maps.append(m)
    import os
    trace = bool(os.environ.get("KERNEL_TRACE"))
    if trace:
        import types
        import ctypes
        if "antenv.axon_hooks" not in sys.modules:
            try:
                import antenv
                from trn_agent_boot.trn_boot import _ntff_profile_via_ctypes
                mod = types.ModuleType("antenv.axon_hooks")
                hook = [_ntff_profile_via_ctypes("/opt/axon/libaxon_pjrt.so")]
                mod.set_axon_ntff_profile_hook = lambda h: hook.__setitem__(0, h)
                mod.get_axon_ntff_profile_hook = lambda: hook[0]
                sys.modules["antenv.axon_hooks"] = mod
                antenv.axon_hooks = mod
            except Exception:
                trace = False
    res = run_bass_kernel_spmd(nc, in_maps, core_ids=list(range(NCORE)),
                               trace=trace,
                               tmpdir=os.environ.get("KERNEL_TRACE_DIR"))
    if trace and res.exec_time_ns is not None:
        print(f"HW exec time: {res.exec_time_ns} ns")
    y_all = np.zeros(C, np.float32)
    p_all = np.zeros(C, np.float32)
    ncc = cm["ncc"]
    p_lin = res.results[0]["p_out"].ravel()   # global order: k, p, c
    for k in range(NCORE):
        sm = cdata[k]["slotmap"]
        valid = sm >= 0
        j = np.nonzero(valid)[0]              # slot j = c*128 + p
        yk = res.results[k]["y_out"]          # [128, NCC] -> value at [j%128, j//128]
        y_all[sm[valid]] = yk[j % P, j // P]
        gs = k * cm["cslot"] + (j % P) * ncc + (j // P)
        p_all[sm[valid]] = p_lin[gs]
    return y_all[:, None], p_all[:, None]


# revision 18
# speedup vs baseline: 1.5220x; 1.0752x over previous
"""Trainium2 Bass kernel for nn_PolicyNetwork3 (2-layer GraphSAGE + edge-MLP).

Design (8 NeuronCores, SPMD single NEFF):
- dst-sharded aggregation: core k owns node block [6272k, 6272k+6272).
- Node tables stored PIECE-MAJOR in one concatenated [50176, 128] table;
  the inter-layer AllGather streams piece-wise into row slices of it,
  overlapped with the window loop. Gather indices are SIGNED int16 offsets
  from the table midpoint (the Q7 address math sign-extends), so one call
  covers the whole table.
- Edges bucketed per 128-dst window; ONE dma_gather call per window
  (amortizes the ~3us per-call Q7 overhead). Every call's last index is
  kept non-negative so the ucode's trailing-negative trim never fires.
- segment-sum per window via one-hot matmuls accumulating in PSUM
  (one-hot weighted by 1/max(deg,1), built on DVE).
- Linear+BN (folded) fused per window, flipped to produce [feat, node];
  layer 1 fuses the candidate-MLP first-layer projection (gq tables).
- Candidate MLP runs in 512-candidate tiles pipelined with the u/v
  gathers; global softmax on-device after an AllGather of y.
"""

import sys

sys.path.insert(0, "/opt/trn_rl_repo")
sys.path.insert(0, "/root/.axon_site")

import numpy as np

import concourse.bacc as bacc
import concourse.bass as bass
import concourse.bass_isa as bass_isa
import concourse.mybir as mybir
import concourse.tile as tile
from concourse import library_config
from concourse.bass_utils import run_bass_kernel_spmd

P = 128
N, E, C = 50000, 800000, 100000
D = 128
NCORE = 8
NSH = 6272          # nodes per core shard (49 blocks of 128)
NTOT = NSH * NCORE  # 50176 padded node table
NWIN = NSH // P     # 49 windows (= 128-node blocks)
HALF = NTOT // 2    # gather base row (signed idx16 offsets from here)
CSH = C // NCORE    # 12500 candidates per core
GMAX = 2560         # max idxs per dma_gather call
TGRP = 4            # candidate chunks per MLP tile (512 cands)
NPIECE = 4
PIECE_BLK = [13, 12, 12, 12]          # blocks per piece
PIECE_R0 = [0, 1664, 3200, 4736]      # first local row of each piece
PIECE_ROWS = [1664, 1536, 1536, 1536]
PIECE_CATB = [0, 13312, 25600, 37888]  # piece base row in the cat table
BN_EPS = 1e-5
SLOPE = 0.01
F32 = mybir.dt.float32
BF16 = mybir.dt.bfloat16
I16 = mybir.dt.int16
AF = mybir.ActivationFunctionType
ALU = mybir.AluOpType


def _wrap16(idx_lin):
    """[n] -> [128, n/16] int16 in the dma_gather wrapped+replicated layout."""
    n = idx_lin.shape[0]
    assert n % 16 == 0
    w = idx_lin.reshape(n // 16, 16).T.astype(np.int16)
    return np.tile(w, (8, 1)).copy()


def gidx_to_cols(arr):
    """[nslot] -> [128, nchunk] with slot i at [i%128, i//128]."""
    n = arr.shape[0]
    return arr.reshape(n // P, P).T.copy()


def _cat_row(src):
    """global node id -> row in the piece-major cat table."""
    src = np.asarray(src)
    k = src // NSH
    r = src % NSH
    p = np.searchsorted(np.asarray(PIECE_R0[1:]), r, side="right")
    return (np.asarray(PIECE_CATB)[p] + k * np.asarray(PIECE_ROWS)[p]
            + (r - np.asarray(PIECE_R0)[p]))


def _split_calls(p0, p1):
    """slot range -> near-equal calls of <= GMAX idxs (multiples of 128)."""
    total = p1 - p0
    ncall = -(-total // GMAX)
    per = -(-(total // P) // ncall) * P
    calls = []
    q = p0
    while q < p1:
        n = min(per, p1 - q)
        calls.append((q, n))
        q += n
    return calls


def _guard_calls(idxmats, calls):
    """Ensure the last slot of every call has all idx matrices >= 0 there,
    swapping in a suitable slot from the same call (or whole range)."""
    nmat = len(idxmats)
    ok = np.ones(idxmats[0].shape[0], bool)
    for m in idxmats:
        ok &= (m >= 0)
    lasts = {q + n - 1 for (q, n) in calls}
    for (q, n) in calls:
        last = q + n - 1
        if ok[last]:
            continue
        cand = [j for j in range(q, q + n - 1) if ok[j] and j not in lasts]
        assert cand, "no non-negative slot available for call guard"
        j = cand[-1]
        for m in idxmats:
            m[last], m[j] = m[j], m[last]
    return idxmats


def _prep_edges(src, dst, invdeg):
    """Per-window buckets of 128-edge chunks, uniform across cores."""
    core = np.minimum(dst // NSH, NCORE - 1)
    winl = (dst - core * NSH) // P
    g = _cat_row(src)
    key = core * NWIN + winl
    nbkt = NCORE * NWIN
    order = np.argsort(key, kind="stable")
    cnt = np.bincount(key, minlength=nbkt).reshape(NCORE, NWIN)
    nch_u = (-(-cnt // P)).max(axis=0)       # [NWIN] uniform chunk counts
    run_off = np.zeros(NWIN + 1, np.int64)
    np.cumsum(nch_u, out=run_off[1:])
    tot_ch = int(run_off[-1])
    nslot = tot_ch * P

    gidx = np.zeros((NCORE, nslot), np.int16)        # pads read row HALF
    dstloc = np.full((NCORE, nslot), -5.0, np.float32)
    val = np.zeros((NCORE, nslot), np.float32)
    bstart = np.zeros(nbkt + 1, np.int64)
    np.cumsum(np.bincount(key, minlength=nbkt), out=bstart[1:])
    run_calls = [_split_calls(int(run_off[w]) * P, int(run_off[w + 1]) * P)
                 for w in range(NWIN)]
    for k in range(NCORE):
        for w in range(NWIN):
            b = k * NWIN + w
            e0, e1 = bstart[b], bstart[b + 1]
            n = e1 - e0
            if n == 0:
                continue
            sl = order[e0:e1]
            sl = sl[np.argsort(g[sl], kind="stable")]
            pos = int(run_off[w]) * P
            gidx[k, pos : pos + n] = (g[sl] - HALF).astype(np.int16)
            dstloc[k, pos : pos + n] = (dst[sl] % P).astype(np.float32)
            val[k, pos : pos + n] = invdeg[dst[sl]]
        # keep last slot of every call non-negative (trim guard); swap
        # dstloc/val along with the index
        for w in range(NWIN):
            calls = run_calls[w]
            lasts = [q + n - 1 for (q, n) in calls]
            for (q, n) in calls:
                last = q + n - 1
                if gidx[k, last] >= 0:
                    continue
                p0, p1 = int(run_off[w]) * P, int(run_off[w + 1]) * P
                cand = [j for j in range(p1 - 1, p0 - 1, -1)
                        if gidx[k, j] >= 0 and j not in lasts]
                assert cand, "no non-negative index in window bucket"
                j = cand[0]
                for m in (gidx, dstloc, val):
                    m[k, last], m[k, j] = m[k, j], m[k, last]
    meta = dict(run_off=run_off, run_nch=[int(x) for x in nch_u],
                tot_ch=tot_ch, nslot=nslot, run_calls=run_calls)
    data = [dict(gidx=_wrap16(gidx[k]),
                 dstloc=gidx_to_cols(dstloc[k]),
                 val=gidx_to_cols(val[k])) for k in range(NCORE)]
    return meta, data


def _prep_cands(cand_u, cand_v, cand_feat):
    """Shard candidates per core, sort by u row, pad to uniform chunks."""
    gu = _cat_row(cand_u) - HALF
    gv = _cat_row(cand_v) - HALF
    ncc = -(-CSH // P)
    cslot = ncc * P
    cu = np.zeros((NCORE, cslot), np.int16)
    cv = np.zeros((NCORE, cslot), np.int16)
    ft = np.zeros((NCORE, cslot), np.float32)
    mask = np.full((NCORE, cslot), -1e30, np.float32)
    slotmap = np.full((NCORE, cslot), -1, np.int64)
    calls = _split_calls(0, cslot)
    for k in range(NCORE):
        ids = np.arange(k * CSH, (k + 1) * CSH)
        ids = ids[np.argsort(gu[ids], kind="stable")]
        cu[k, :CSH] = gu[ids].astype(np.int16)
        cv[k, :CSH] = gv[ids].astype(np.int16)
        ft[k, :CSH] = cand_feat[ids, 0]
        mask[k, :CSH] = 0.0
        slotmap[k, :CSH] = ids
        # joint guard: last slot of each call needs cu>=0 AND cv>=0
        both = (cu[k] >= 0) & (cv[k] >= 0)
        lasts = [q + n - 1 for (q, n) in calls]
        for (q, n) in calls:
            last = q + n - 1
            if both[last]:
                continue
            cand = [j for j in range(q + n - 2, q - 1, -1)
                    if both[j] and j not in lasts]
            if not cand:
                cand = [j for j in range(cslot - 1, -1, -1)
                        if both[j] and j not in lasts]
            assert cand, "no jointly non-negative candidate for call guard"
            j = cand[0]
            for m in (cu, cv, ft, mask):
                m[k, last], m[k, j] = m[k, j], m[k, last]
            slotmap[k, last], slotmap[k, j] = slotmap[k, j], slotmap[k, last]
            both[last], both[j] = both[j], both[last]
    meta = dict(ncc=ncc, cslot=cslot, calls=calls)
    data = [dict(cu=_wrap16(cu[k]), cv=_wrap16(cv[k]),
                 feat=gidx_to_cols(ft[k]), mask=gidx_to_cols(mask[k]),
                 slotmap=slotmap[k]) for k in range(NCORE)]
    return meta, data


def _build_nc(em, cm):
    nc = bacc.Bacc("TRN2", target_bir_lowering=False, debug=False,
                   num_devices=NCORE, num_swdge_queues=4)
    f32 = F32
    TOTCH, NSLOT = em["tot_ch"], em["nslot"]
    NCC, CSLOT = cm["ncc"], cm["cslot"]

    # ---- external inputs ----
    xcat = nc.dram_tensor("xcat", [NTOT, D], BF16, kind="ExternalInput")
    xT = nc.dram_tensor("xT", [P, NSH], f32, kind="ExternalInput")
    gidx = nc.dram_tensor("gidx", [P, NSLOT // 16], I16, kind="ExternalInput")
    dstloc = nc.dram_tensor("dstloc", [P, TOTCH], f32, kind="ExternalInput")
    val = nc.dram_tensor("val", [P, TOTCH], f32, kind="ExternalInput")
    wself = [nc.dram_tensor(f"wself{l}", [D, D], f32, kind="ExternalInput") for l in range(2)]
    wneigh = [nc.dram_tensor(f"wneigh{l}", [D, D], f32, kind="ExternalInput") for l in range(2)]
    crow = [nc.dram_tensor(f"crow{l}", [1, D], f32, kind="ExternalInput") for l in range(2)]
    iota = nc.dram_tensor("iota", [P, P], f32, kind="ExternalInput")
    ident = nc.dram_tensor("ident", [P, P], f32, kind="ExternalInput")
    onesr = nc.dram_tensor("onesr", [1, P], f32, kind="ExternalInput")
    abmat = nc.dram_tensor("abmat", [D, D], f32, kind="ExternalInput")
    gqbias = nc.dram_tensor("gqbias", [1, D], f32, kind="ExternalInput")
    mw0r = nc.dram_tensor("mw0r", [P, 64], f32, kind="ExternalInput")
    mw1b = nc.dram_tensor("mw1b", [65, 64], f32, kind="ExternalInput")
    mw2b = nc.dram_tensor("mw2b", [65, 1], f32, kind="ExternalInput")
    cu = nc.dram_tensor("cu", [P, CSLOT // 16], I16, kind="ExternalInput")
    cv = nc.dram_tensor("cv", [P, CSLOT // 16], I16, kind="ExternalInput")
    feat = nc.dram_tensor("feat", [P, NCC], f32, kind="ExternalInput")
    maskr = nc.dram_tensor("maskr", [P, NCC], f32, kind="ExternalInput")
    # ---- outputs ----
    y_out = nc.dram_tensor("y_out", [P, NCC], f32, kind="ExternalOutput")
    p_out = nc.dram_tensor("p_out", [P, NCORE * CSLOT // P], f32, kind="ExternalOutput")
    # ---- internal DRAM ----
    hshp = [nc.dram_tensor(f"hshp{p}", [PIECE_ROWS[p], D], BF16, kind="Internal")
            for p in range(NPIECE)]
    hcat = nc.dram_tensor("hcat", [NTOT, D], BF16, kind="Internal", addr_space="Shared")
    gqshp = [nc.dram_tensor(f"gqshp{p}", [PIECE_ROWS[p], D], BF16, kind="Internal")
             for p in range(NPIECE)]
    gqcat = nc.dram_tensor("gqcat", [NTOT, D], BF16, kind="Internal", addr_space="Shared")
    ysh = nc.dram_tensor("ysh", [P, NCC], f32, kind="Internal")
    yfull = nc.dram_tensor("yfull", [NCORE * P, NCC], f32, kind="Internal", addr_space="Shared")

    rg = [list(range(NCORE))]
    run_nch, run_calls = em["run_nch"], em["run_calls"]
    pend = np.cumsum(PIECE_BLK) - 1          # windows 12, 24, 36, 48

    with tile.TileContext(nc) as tc:
        with (
            tc.tile_pool(name="const", bufs=1) as cp,
            tc.tile_pool(name="big", bufs=1) as bp,
            tc.tile_pool(name="msgs", bufs=3) as mp,
            tc.tile_pool(name="oh", bufs=4) as ohp,
            tc.tile_pool(name="wrk", bufs=4) as wp,
            tc.tile_pool(name="zt", bufs=3) as zp,
            tc.tile_pool(name="ps_run", bufs=2, space="PSUM") as ps_run,
            tc.tile_pool(name="ps_t", bufs=2, space="PSUM") as ps_t,
            tc.tile_pool(name="ps_h", bufs=2, space="PSUM") as ps_h,
        ):
            nc.gpsimd.load_library(library_config.mlp)

            def load(pool, t, shape=None):
                tl = pool.tile(shape or list(t.shape), t.dtype, tag=t.name)
                nc.sync.dma_start(tl[:], t[:])
                return tl

            gidx_t = load(cp, gidx)
            dstloc_t = load(cp, dstloc)
            val_t = load(cp, val)
            iota_t = load(cp, iota)
            ident_t = load(cp, ident)
            onesr_t = load(cp, onesr)
            wself_t = [load(cp, w) for w in wself]
            wneigh_t = [load(cp, w) for w in wneigh]
            crow_t = [load(cp, w) for w in crow]
            abmat_t = load(cp, abmat)
            gqbias_t = load(cp, gqbias)
            mw0r_t = load(cp, mw0r)
            mw1b_t = load(cp, mw1b)
            mw2b_t = load(cp, mw2b)
            cu_t = load(cp, cu)
            cv_t = load(cp, cv)
            feat_t = load(cp, feat)
            mask_t = load(cp, maskr)

            curT = bp.tile([P, NSH], f32, tag="curT")
            nxtT = bp.tile([P, NSH], f32, tag="nxtT")
            nc.sync.dma_start(curT[:], xT[:])

            # zero gather bufs once: short calls leave stale tail chunks
            # whose one-hot is all-zero; 0*NaN would poison the psum
            for _ in range(3):
                g0 = mp.tile([P, GMAX // P, P], BF16, tag="g")
                nc.vector.memset(g0[:], 0.0)
            ut = bp.tile([P, NCC, D], BF16, tag="ut")
            vt = bp.tile([P, NCC, D], BF16, tag="vt")

            qrr = [0]
            for layer in range(2):
                tab = xcat if layer == 0 else hcat
                for w in range(NWIN):
                    wch = run_nch[w]
                    aggw = wp.tile([P, P], f32, tag="aggw")
                    if wch == 0:
                        nc.vector.memset(aggw[:], 0.0)
                    else:
                        ps = ps_run.tile([P, P], f32, tag="psw")
                        jw = 0
                        for (s0, n_idx) in run_calls[w]:
                            ncall = n_idx // P
                            g = mp.tile([P, GMAX // P, P], BF16, tag="g")
                            nc.gpsimd.dma_gather(
                                g[:, :ncall, :], tab[HALF:NTOT, :],
                                gidx_t[:, s0 // 16 : (s0 + n_idx) // 16],
                                n_idx, n_idx, P, single_packet=False,
                                queue_num=qrr[0])
                            qrr[0] = (qrr[0] + 1) % 4
                            for cc in range(ncall):
                                ch = s0 // P + cc
                                oh = ohp.tile([P, P], BF16, tag="oh")
                                nc.vector.tensor_scalar(
                                    oh[:], iota_t[:], dstloc_t[:, ch : ch + 1],
                                    val_t[:, ch : ch + 1], ALU.is_equal, ALU.mult)
                                nc.tensor.matmul(ps[:], lhsT=g[:, cc, :], rhs=oh[:],
                                                 start=(jw == 0), stop=(jw == wch - 1))
                                jw += 1
                        nc.scalar.activation(aggw[:], ps[:], AF.Copy)
                    nb = w * P
                    ph = ps_h.tile([P, P], f32, tag="ph")
                    nc.tensor.matmul(ph[:], lhsT=wneigh_t[layer][:], rhs=aggw[:],
                                     start=True, stop=False)
                    nc.tensor.matmul(ph[:], lhsT=wself_t[layer][:],
                                     rhs=curT[:, nb : nb + P], start=False, stop=False)
                    nc.tensor.matmul(ph[:], lhsT=crow_t[layer][:], rhs=onesr_t[:],
                                     start=False, stop=True)
                    tmp = wp.tile([P, P], f32, tag="tmp")
                    nc.vector.tensor_scalar(tmp[:], ph[:], SLOPE, None, ALU.mult)
                    nc.vector.tensor_tensor(nxtT[:, nb : nb + P], ph[:], tmp[:], ALU.max)
                    wp_ = next(i for i in range(NPIECE)
                               if PIECE_R0[i] <= nb < PIECE_R0[i] + PIECE_BLK[i] * P)
                    rb = nb - PIECE_R0[wp_]
                    if layer == 0:
                        pt = ps_t.tile([P, P], f32, tag="pt")
                        nc.tensor.transpose(pt[:], nxtT[:, nb : nb + P], ident_t[:])
                        hb = wp.tile([P, P], BF16, tag="hb")
                        nc.scalar.activation(hb[:], pt[:], AF.Copy)
                        nc.sync.dma_start(hshp[wp_][rb : rb + P, :], hb[:])
                    else:
                        pg = ps_h.tile([P, P], f32, tag="ph")
                        nc.tensor.matmul(pg[:], lhsT=nxtT[:, nb : nb + P],
                                         rhs=abmat_t[:], start=True, stop=False)
                        nc.tensor.matmul(pg[:], lhsT=onesr_t[:], rhs=gqbias_t[:],
                                         start=False, stop=True)
                        gb = wp.tile([P, P], BF16, tag="gb")
                        nc.scalar.activation(gb[:], pg[:], AF.Copy)
                        nc.sync.dma_start(gqshp[wp_][rb : rb + P, :], gb[:])
                    if w in pend:
                        pi = int(np.nonzero(pend == w)[0][0])
                        cb0 = PIECE_CATB[pi]
                        cb1 = cb0 + NCORE * PIECE_ROWS[pi]
                        if layer == 0:
                            nc.gpsimd.collective_compute(
                                "AllGather", ALU.bypass, replica_groups=rg,
                                ins=[hshp[pi][:].opt()], outs=[hcat[cb0:cb1, :].opt()])
                        else:
                            nc.gpsimd.collective_compute(
                                "AllGather", ALU.bypass, replica_groups=rg,
                                ins=[gqshp[pi][:].opt()], outs=[gqcat[cb0:cb1, :].opt()])
                if layer == 0:
                    curT, nxtT = nxtT, curT

            # ---- candidate gathers (u then v, interleaved by slot) ----
            for (s0, n_idx) in cm["calls"]:
                for (tl, idx_t) in ((ut, cu_t), (vt, cv_t)):
                    nc.gpsimd.dma_gather(
                        tl[:, s0 // P : (s0 + n_idx) // P, :],
                        gqcat[HALF:NTOT, :],
                        idx_t[:, s0 // 16 : (s0 + n_idx) // 16],
                        n_idx, n_idx, D, single_packet=False,
                        queue_num=qrr[0])
                    qrr[0] = (qrr[0] + 1) % 4

            # ---- candidate MLP in 512-cand tiles, 2-stage pipelined ----
            ycol = bp.tile([P, NCC], f32, tag="ycol")

            def mlp_stage_a(t0):
                tn = min(TGRP, NCC - t0)
                z1 = zp.tile([P, TGRP, 64], f32, tag="z1")
                for c in range(tn):
                    nc.vector.tensor_scalar(z1[:, c, :], mw0r_t[:],
                                            feat_t[:, t0 + c : t0 + c + 1],
                                            None, ALU.mult)
                nc.vector.tensor_tensor(z1[:, :tn, :], z1[:, :tn, :],
                                        ut[:, t0 : t0 + tn, 0:64], ALU.add)
                nc.vector.tensor_tensor(z1[:, :tn, :], z1[:, :tn, :],
                                        vt[:, t0 : t0 + tn, 64:128], ALU.add)
                zs = zp.tile([P, TGRP, 64], f32, tag="zs")
                nc.vector.tensor_scalar(zs[:, :tn, :], z1[:, :tn, :], SLOPE,
                                        None, ALU.mult)
                nc.vector.tensor_tensor(z1[:, :tn, :], z1[:, :tn, :],
                                        zs[:, :tn, :], ALU.max)
                z1t = zp.tile([65, TGRP * P], f32, tag="z1t")
                nc.vector.memset(z1t[64:65, :], 1.0)
                for c in range(tn):
                    pt2 = ps_t.tile([64, P], f32, tag="pt")
                    nc.tensor.transpose(pt2[:], z1[:, c, :], ident_t[:])
                    nc.scalar.activation(z1t[0:64, c * P : (c + 1) * P], pt2[:], AF.Copy)
                return t0, tn, z1t

            def mlp_stage_b(st):
                t0, tn, z1t = st
                ps2 = ps_run.tile([64, TGRP * P], f32, tag="ps2")
                nc.tensor.matmul(ps2[:, : tn * P], lhsT=mw1b_t[:],
                                 rhs=z1t[:, : tn * P], start=True, stop=True)
                z2t = zp.tile([65, TGRP * P], f32, tag="z2t")
                nc.vector.memset(z2t[64:65, :], 1.0)
                nc.vector.tensor_scalar(z2t[0:64, : tn * P], ps2[:, : tn * P],
                                        SLOPE, None, ALU.mult)
                nc.vector.tensor_tensor(z2t[0:64, : tn * P], z2t[0:64, : tn * P],
                                        ps2[:, : tn * P], ALU.max)
                for c in range(tn):
                    py = ps_h.tile([P, 1], f32, tag="ph")
                    nc.tensor.matmul(py[:], lhsT=z2t[:, c * P : (c + 1) * P],
                                     rhs=mw2b_t[:], start=True, stop=True)
                    nc.scalar.activation(ycol[:, t0 + c : t0 + c + 1], py[:], AF.Copy)

            pending = None
            for t0 in range(0, NCC, TGRP):
                st = mlp_stage_a(t0)
                if pending is not None:
                    mlp_stage_b(pending)
                pending = st
            mlp_stage_b(pending)

            nc.sync.dma_start(y_out[:], ycol[:])
            ym = wp.tile([P, NCC], f32, tag="ym")
            nc.vector.tensor_tensor(ym[:], ycol[:], mask_t[:], ALU.add)
            nc.sync.dma_start(ysh[:], ym[:])
            nc.gpsimd.collective_compute(
                "AllGather", ALU.bypass, replica_groups=rg,
                ins=[ysh[:].opt()], outs=[yfull[:].opt()])
            # ---- softmax ----
            ncols = NCORE * CSLOT // P
            yf = bp.tile([P, ncols], f32, tag="yf")
            nc.sync.dma_start(yf[:], yfull[:].rearrange("a b -> (a b)")
                              .rearrange("(p c) -> p c", p=P))
            rmax = wp.tile([P, 1], f32, tag="rmax")
            nc.vector.tensor_reduce(rmax[:], yf[:], mybir.AxisListType.X, ALU.max)
            gmax = wp.tile([P, 1], f32, tag="gmax")
            nc.gpsimd.partition_all_reduce(gmax[:], rmax[:], P,
                                           bass_isa.ReduceOp.max)
            ngmax = wp.tile([P, 1], f32, tag="ngmax")
            nc.vector.tensor_scalar(ngmax[:], gmax[:], -1.0, None, ALU.mult)
            ef = bp.tile([P, ncols], f32, tag="ef")
            se = wp.tile([P, 1], f32, tag="se")
            nc.scalar.activation(ef[:], yf[:], AF.Exp, bias=ngmax[:, 0:1],
                                 accum_out=se[:])
            stot = wp.tile([P, 1], f32, tag="stot")
            nc.gpsimd.partition_all_reduce(stot[:], se[:], P, bass_isa.ReduceOp.add)
            invs = wp.tile([P, 1], f32, tag="invs")
            nc.vector.reciprocal(invs[:], stot[:])
            pf = bp.tile([P, ncols], f32, tag="pf")
            nc.vector.tensor_scalar(pf[:], ef[:], invs[:, 0:1], None, ALU.mult)
            nc.sync.dma_start(p_out[:], pf[:])
    nc.compile()
    return nc


def kernel(x, src, dst, cand_u, cand_v, cand_feat,
           w_self0, w_neigh0, b0, gamma0, beta0, rm0, rv0,
           w_self1, w_neigh1, b1, gamma1, beta1, rm1, rv1,
           mw0, mb0, mw1, mb1, mw2, mb2):
    x = np.asarray(x, np.float32)
    src = np.asarray(src, np.int64)
    dst = np.asarray(dst, np.int64)
    cand_u = np.asarray(cand_u, np.int64)
    cand_v = np.asarray(cand_v, np.int64)
    cand_feat = np.asarray(cand_feat, np.float32)

    deg = np.bincount(dst, minlength=N).astype(np.float32)
    invdeg = 1.0 / np.maximum(deg, 1.0)
    em, edata = _prep_edges(src, dst, invdeg)
    cm, cdata = _prep_cands(cand_u, cand_v, cand_feat)

    xpad = np.zeros((NTOT, D), np.float32)
    xpad[:N] = x
    iota = np.tile(np.arange(P, dtype=np.float32), (P, 1))
    ident = np.eye(P, dtype=np.float32)
    onesr = np.ones((1, P), np.float32)

    com = {"iota": iota, "ident": ident, "onesr": onesr}
    # piece-major concatenated x table
    import ml_dtypes
    xsh = xpad.reshape(NCORE, NSH, D)
    xcat = np.zeros((NTOT, D), ml_dtypes.bfloat16)
    for p in range(NPIECE):
        r0, rows, cb = PIECE_R0[p], PIECE_ROWS[p], PIECE_CATB[p]
        xcat[cb : cb + NCORE * rows] = xsh[:, r0 : r0 + rows, :].reshape(-1, D)
    com["xcat"] = xcat
    for l, (ws, wn, b, ga, be, rme, rve) in enumerate(
        ((w_self0, w_neigh0, b0, gamma0, beta0, rm0, rv0),
         (w_self1, w_neigh1, b1, gamma1, beta1, rm1, rv1))):
        a = (ga / np.sqrt(rve + BN_EPS)).astype(np.float32)
        com[f"wself{l}"] = (ws * a[None, :]).astype(np.float32)
        com[f"wneigh{l}"] = (wn * a[None, :]).astype(np.float32)
        com[f"crow{l}"] = (a * (b - rme) + be).astype(np.float32)[None, :]
    com["abmat"] = np.concatenate(
        [np.asarray(mw0[0:128], np.float32), np.asarray(mw0[128:256], np.float32)], axis=1)
    com["gqbias"] = np.concatenate(
        [np.zeros(64, np.float32), np.asarray(mb0, np.float32)])[None, :]
    com["mw0r"] = np.tile(np.asarray(mw0[256], np.float32), (P, 1))
    com["mw1b"] = np.concatenate(
        [np.asarray(mw1, np.float32), np.asarray(mb1, np.float32)[None, :]], axis=0)
    com["mw2b"] = np.concatenate(
        [np.asarray(mw2, np.float32),
         np.asarray(mb2, np.float32).reshape(1, 1)], axis=0)

    nc = _build_nc(em, cm)
    in_maps = []
    for k in range(NCORE):
        m = dict(com)
        m["xT"] = xpad[k * NSH : (k + 1) * NSH].T.copy()
        m["gidx"] = edata[k]["gidx"]
        m["dstloc"] = edata[k]["dstloc"]
        m["val"] = edata[k]["val"]
        m["cu"] = cdata[k]["cu"]
        m["cv"] = cdata[k]["cv"]
        m["feat"] = cdata[k]["feat"]
        m["maskr"] = cdata[k]["mask"]
        in_maps.append(m)
    import os
    trace = bool(os.environ.get("KERNEL_TRACE"))
    if trace:
        import types
        import ctypes
        if "antenv.axon_hooks" not in sys.modules:
            try:
                import antenv
                from trn_agent_boot.trn_boot import _ntff_profile_via_ctypes
                mod = types.ModuleType("antenv.axon_hooks")
                hook = [_ntff_profile_via_ctypes("/opt/axon/libaxon_pjrt.so")]
                mod.set_axon_ntff_profile_hook = lambda h: hook.__setitem__(0, h)
                mod.get_axon_ntff_profile_hook = lambda: hook[0]
                sys.modules["antenv.axon_hooks"] = mod
                antenv.axon_hooks = mod
            except Exception:
                trace = False
    res = run_bass_kernel_spmd(nc, in_maps, core_ids=list(range(NCORE)),
                               trace=trace,
                               tmpdir=os.environ.get("KERNEL_TRACE_DIR"))
    if trace and res.exec_time_ns is not None:
        print(f"HW exec time: {res.exec_time_ns} ns")
    y_all = np.zeros(C, np.float32)
    p_all = np.zeros(C, np.float32)
    ncc = cm["ncc"]
    p_lin = res.results[0]["p_out"].ravel()   # global order: k, p, c
    for k in range(NCORE):
        sm = cdata[k]["slotmap"]
        valid = sm >= 0
        j = np.nonzero(valid)[0]              # slot j = c*128 + p
        yk = res.results[k]["y_out"]          # [128, NCC] -> value at [j%128, j//128]
        y_all[sm[valid]] = yk[j % P, j // P]
        gs = k * cm["cslot"] + (j % P) * ncc + (j // P)
        p_all[sm[valid]] = p_lin[gs]
    return y_all[:, None], p_all[:, None]
